# revision 1
# baseline (speedup 1.0000x reference)
"""HardAttentionMemoryAE Trainium2 kernel.

Data-parallel over 8 NeuronCores: x sharded along batch, weights + 50x128
memory bank replicated. Per core the pipeline runs in "transposed
activation" layout (features on partitions, rows on the free dim) so every
matmul contracts along partitions, with a row-major detour for the top-k
masking (per-row ops need rows on partitions).

Numerics: all large matmuls use float32r (fp32 rounded to 11 mantissa
bits, full PE rate at N>=256). Offline simulation vs the fp32 reference:
L2 rel err 3.3e-5; top-5 selection flips on 84/65536 rows with ~8.5e-4
row-level error (flips only occur at near-ties where impact is bounded).
"""
import numpy as np
import concourse.bass as bass
import concourse.mybir as mybir
from concourse import bacc
from concourse.tile import TileContext
from concourse.masks import make_identity
from concourse.bass_utils import run_bass_kernel_spmd

F32 = mybir.dt.float32
F32R = mybir.dt.float32r
AF = mybir.ActivationFunctionType
ALU = mybir.AluOpType

B_FULL = 65536
D = 784          # input dim
E = 128          # embed dim
M = 50           # memory slots
H = 256          # hidden
N_CORES = 8
SLAB = 512       # rows per slab (4 row-tiles of 128)
NHALF = 392      # final matmul N split (per PSUM bank, >=256 keeps f32r rate)

TRACE = False    # set by test harness for profiling runs
STAGE = None     # debug: truncate slab body after stage N
SETUPN = 99      # debug: include setup items < SETUPN


class _SetupCut(Exception):
    pass


def _build(rows: int, n_cores: int, bias_mm: bool):
    nc = bacc.Bacc(
        "TRN2", target_bir_lowering=False, debug=False,
        enable_asserts=True, num_devices=n_cores
    )
    x = nc.dram_tensor("x", [rows, D], F32, kind="ExternalInput")
    W1 = nc.dram_tensor("W1", [D, H], F32, kind="ExternalInput")
    b1 = nc.dram_tensor("b1", [H], F32, kind="ExternalInput")
    W2 = nc.dram_tensor("W2", [H, E], F32, kind="ExternalInput")
    b2 = nc.dram_tensor("b2", [E], F32, kind="ExternalInput")
    mem = nc.dram_tensor("memory", [M, E], F32, kind="ExternalInput")
    W3 = nc.dram_tensor("W3", [E, H], F32, kind="ExternalInput")
    b3 = nc.dram_tensor("b3", [H], F32, kind="ExternalInput")
    W4 = nc.dram_tensor("W4", [H, D], F32, kind="ExternalInput")
    b4 = nc.dram_tensor("b4", [D], F32, kind="ExternalInput")
    y = nc.dram_tensor("y", [rows, D], F32, kind="ExternalOutput")

    n_slabs = rows // SLAB
    # x col chunks for the transpose: 6 aligned chunks + one overlapping
    # final chunk [656, 784) whose first 112 weight rows are zeroed.
    COFF = [0, 128, 256, 384, 512, 640, D - 128]

    x_r = x[:].rearrange("(s t p) c -> s p t c", p=128, t=4)
    y_r = y[:].rearrange("(s t p) c -> s p t c", p=128, t=4)

    with TileContext(nc) as tc:
        with (
            tc.tile_pool(name="const", bufs=1) as cpool,
            tc.tile_pool(name="xr", bufs=2) as xr_pool,
            tc.tile_pool(name="xT", bufs=2) as xT_pool,
            tc.tile_pool(name="hT", bufs=2) as hT_pool,
            tc.tile_pool(name="zT", bufs=2) as zT_pool,
            tc.tile_pool(name="small", bufs=2) as sm_pool,
            tc.tile_pool(name="xout", bufs=2) as xo_pool,
            tc.tile_pool(name="pbig", bufs=3, space="PSUM") as pbig,
            tc.tile_pool(name="pmid", bufs=2, space="PSUM") as pmid,
            tc.tile_pool(name="psml", bufs=1, space="PSUM") as psml,
            tc.tile_pool(name="px", bufs=2, space="PSUM") as pxp,
        ):
            try:
                # ---------------- one-time setup ----------------
                if SETUPN < 1: raise _SetupCut()
                W1sb = cpool.tile([128, 7 * H], F32R)
                zpad = cpool.tile([128, H], F32)
                nc.vector.memset(zpad[:], 0.0)
                nc.scalar.copy(W1sb[:, 6 * H:], zpad[:])
                for c in range(6):
                    nc.gpsimd.dma_start(
                        W1sb[:, c * H:(c + 1) * H], W1[COFF[c]:COFF[c] + 128, :]
                    )
                nc.gpsimd.dma_start(W1sb[112:128, 6 * H:7 * H], W1[768:D, :])
                if SETUPN < 2: raise _SetupCut()
                W2sb = cpool.tile([128, 2 * E], F32R)
                for m in range(2):
                    nc.gpsimd.dma_start(
                        W2sb[:, m * E:(m + 1) * E], W2[m * 128:(m + 1) * 128, :]
                    )
                if SETUPN < 3: raise _SetupCut()
                W3sb = cpool.tile([128, H], F32R)
                nc.gpsimd.dma_start(W3sb[:], W3[:])
                if SETUPN < 4: raise _SetupCut()
                W4sb = cpool.tile([128, 2 * D], F32R)
                for k in range(2):
                    nc.gpsimd.dma_start(
                        W4sb[:, k * D:(k + 1) * D], W4[k * 128:(k + 1) * 128, :]
                    )
                if SETUPN < 5: raise _SetupCut()
                memsb = cpool.tile([M, E], F32R)
                nc.gpsimd.dma_start(memsb[:], mem[:])

                if SETUPN < 6: raise _SetupCut()
                b1sb = cpool.tile([128, 2], F32)
                nc.sync.dma_start(b1sb[:], b1[:].rearrange("(m p) -> p m", p=128))
                b2sb = cpool.tile([128, 1], F32)
                nc.sync.dma_start(b2sb[:], b2[:].rearrange("(p o) -> p o", o=1))
                b3sb = cpool.tile([128, 2], F32)
                nc.sync.dma_start(b3sb[:], b3[:].rearrange("(m p) -> p m", p=128))
                if bias_mm:
                    b4row = cpool.tile([1, D], F32R)
                    nc.gpsimd.dma_start(b4row[:], b4[:].rearrange("(o c) -> o c", o=1))
                    ones_row = cpool.tile([1, 128], F32R)
                    onesr_f = cpool.tile([1, 128], F32)
                    nc.vector.memset(onesr_f[:], 1.0)
                    nc.scalar.copy(ones_row[:], onesr_f[:])

                if SETUPN < 7: raise _SetupCut()
                ident_f = cpool.tile([128, 128], F32)
                make_identity(nc, ident_f[:])
                ident = cpool.tile([128, 128], F32R)
                nc.scalar.copy(ident[:], ident_f[:])

                if SETUPN < 8: raise _SetupCut()
                ones_f = cpool.tile([128, 1], F32)
                nc.vector.memset(ones_f[:], 1.0)
                ones_col = cpool.tile([128, 1], F32R)
                nc.scalar.copy(ones_col[:], ones_f[:])

                if SETUPN < 9: raise _SetupCut()
                # normalized memory, transposed: mem_normT [E, M]
                memf = cpool.tile([M, E], F32)
                nc.sync.dma_start(memf[:], mem[:])
                msq = cpool.tile([M, E], F32)
                nc.scalar.square(msq[:], memf[:])
                mss = cpool.tile([M, 1], F32)
                nc.vector.tensor_reduce(mss[:], msq[:], mybir.AxisListType.X, ALU.add)
                nc.scalar.sqrt(mss[:], mss[:])
                nc.vector.tensor_scalar_max(mss[:], mss[:], 1e-12)
                minv = cpool.tile([M, 1], F32)
                nc.vector.reciprocal(minv[:], mss[:])
                mnorm = cpool.tile([M, E], F32R)
                nc.vector.tensor_scalar_mul(mnorm[:], memf[:], minv[:, 0:1])
                p_mn = psml.tile([128, 512], F32, tag="sml")
                nc.tensor.transpose(p_mn[:E, :M].bitcast(F32R), mnorm[:], ident[:M, :M])
                mnT = cpool.tile([E, M], F32R)
                nc.scalar.copy(mnT[:], p_mn[:E, :M])


            except _SetupCut:
                pass
            # ---------------- steady-state slabs ----------------
            for s in range(n_slabs):
                if STAGE is not None and STAGE < 1:
                    continue
                xr = xr_pool.tile([128, 4, D], F32R, tag="xr")
                nc.gpsimd.dma_start(xr[:], x_r[s])

                # transpose x -> xT chunks [128, 7, 512]
                xT = [xT_pool.tile([128, SLAB], F32R, tag=f"xt{c}",
                                   name=f"xt{c}_{s}")
                      for c in range(7)]
                for c in range(7):
                    ptr = pmid.tile([128, 512], F32, tag="mid")
                    for t in range(4):
                        nc.tensor.transpose(
                            ptr[:, t * 128:(t + 1) * 128].bitcast(F32R),
                            xr[:, t, COFF[c]:COFF[c] + 128],
                            ident[:],
                        )
                    nc.vector.tensor_copy(xT[c][:], ptr[:])

                if STAGE is not None and STAGE < 1:
                    continue
                # phase1: hT = relu(W1.T @ xT + b1)  [2x128, 512]
                hT = hT_pool.tile([128, 1024], F32R, tag="hT")
                for m in range(2):
                    ph = pbig.tile([128, 512], F32, tag="big", name=f"ph{m}_{s}")
                    for c in range(7):
                        nc.tensor.matmul(
                            ph[:],
                            W1sb[:, c * H + m * 128: c * H + m * 128 + 128],
                            xT[c][:],
                            start=(c == 0), stop=(c == 6),
                        )
                    nc.scalar.activation(
                        hT[:, m * 512:(m + 1) * 512], ph[:],
                        AF.Relu, bias=b1sb[:, m:m + 1],
                    )

                if STAGE is not None and STAGE < 2:
                    continue
                # phase2: zT = W2.T @ hT + b2  [128, 512]
                pz = psml.tile([128, 512], F32, tag="sml")
                for m in range(2):
                    nc.tensor.matmul(
                        pz[:], W2sb[:, m * E:(m + 1) * E],
                        hT[:, m * 512:(m + 1) * 512],
                        start=(m == 0), stop=(m == 1),
                    )
                zT = zT_pool.tile([128, SLAB], F32R, tag="zT")
                nc.scalar.activation(zT[:], pz[:], AF.Identity, bias=b2sb[:, 0:1])
                zsq = zT_pool.tile([128, SLAB], F32R, tag="zsq")
                nc.scalar.activation(zsq[:], pz[:], AF.Square, bias=b2sb[:, 0:1])

                if STAGE is not None and STAGE < 3:
                    continue
                # row norms: nsq[1,512] = ones.T @ zsq ; invnorm flip to [128,4]
                pn = psml.tile([128, 512], F32, tag="sml")
                nc.tensor.matmul(pn[:1, :], ones_col[:], zsq[:],
                                 start=True, stop=True)
                nrow = sm_pool.tile([1, SLAB], F32, tag="nrow")
                nc.scalar.sqrt(nrow[:], pn[:1, :])
                nc.vector.tensor_scalar_max(nrow[:], nrow[:], 1e-12)
                invrow = sm_pool.tile([1, SLAB], F32, tag="invrow")
                nc.vector.reciprocal(invrow[:], nrow[:])
                invcol = sm_pool.tile([128, 4], F32, tag="invcol")
                pic = psml.tile([128, 512], F32, tag="sml")
                for t in range(4):
                    nc.tensor.transpose(
                        pic[:, t:t + 1],
                        invrow[:, t * 128:(t + 1) * 128],
                        ident_f[:1, :1],
                    )
                nc.scalar.copy(invcol[:], pic[:, :4])

                if STAGE is not None and STAGE < 4:
                    continue
                # sim + top-5 threshold mask + softmax (row-major detour)
                ps_ = psml.tile([128, 512], F32, tag="sml")
                m8 = sm_pool.tile([128, 32], F32, tag="m8")
                sims = sm_pool.tile([128, 4 * M], F32, tag="sims")
                msk = sm_pool.tile([128, 4 * M], F32, tag="msk")
                pexp = sm_pool.tile([128, 4 * M], F32, tag="pexp")
                den = sm_pool.tile([128, 4], F32, tag="den")
                rden = sm_pool.tile([128, 4], F32, tag="rden")
                attn = sm_pool.tile([128, 4 * M], F32R, tag="attn")
                pat = psml.tile([128, 512], F32, tag="sml")
                for t in range(4):
                    nc.tensor.matmul(
                        ps_[:, t * M:(t + 1) * M],
                        zT[:, t * 128:(t + 1) * 128], mnT[:],
                        start=True, stop=True,
                    )
                    nc.vector.tensor_scalar_mul(
                        sims[:, t * M:(t + 1) * M], ps_[:, t * M:(t + 1) * M],
                        invcol[:, t:t + 1],
                    )
                    nc.vector.max(m8[:, t * 8:(t + 1) * 8], sims[:, t * M:(t + 1) * M])
                    nc.vector.scalar_tensor_tensor(
                        out=msk[:, t * M:(t + 1) * M],
                        in0=sims[:, t * M:(t + 1) * M],
                        scalar=m8[:, t * 8 + 4:t * 8 + 5],
                        in1=sims[:, t * M:(t + 1) * M],
                        op0=ALU.is_ge, op1=ALU.mult,
                    )
                    nc.scalar.activation(
                        pexp[:, t * M:(t + 1) * M], msk[:, t * M:(t + 1) * M],
                        AF.Exp, accum_out=den[:, t:t + 1],
                    )
                nc.vector.reciprocal(rden[:], den[:])
                for t in range(4):
                    nc.vector.tensor_scalar_mul(
                        attn[:, t * M:(t + 1) * M], pexp[:, t * M:(t + 1) * M],
                        rden[:, t:t + 1],
                    )
                    nc.tensor.transpose(
                        pat[:M, t * 128:(t + 1) * 128].bitcast(F32R),
                        attn[:, t * M:(t + 1) * M], ident[:],
                    )
                attnT = sm_pool.tile([M, SLAB], F32R, tag="attnT")
                nc.vector.tensor_copy(attnT[:], pat[:M, :])

                if STAGE is not None and STAGE < 5:
                    continue
                # z_memT = memory.T @ attnT  [128, 512]
                pzm = psml.tile([128, 512], F32, tag="sml")
                nc.tensor.matmul(pzm[:], memsb[:], attnT[:], start=True, stop=True)
                zm = zT_pool.tile([128, SLAB], F32R, tag="zm")
                nc.vector.tensor_copy(zm[:], pzm[:])

                if STAGE is not None and STAGE < 6:
                    continue
                # decoder hidden: dT = relu(W3.T @ zm + b3) [2x128, 512]
                dT = hT_pool.tile([128, 1024], F32R, tag="dT")
                for m in range(2):
                    pd = pbig.tile([128, 512], F32, tag="big", name=f"pd{m}_{s}")
                    nc.tensor.matmul(
                        pd[:], W3sb[:, m * 128:(m + 1) * 128], zm[:],
                        start=True, stop=True,
                    )
                    nc.scalar.activation(
                        dT[:, m * 512:(m + 1) * 512], pd[:],
                        AF.Relu, bias=b3sb[:, m:m + 1],
                    )

                if STAGE is not None and STAGE < 7:
                    continue
                # final: x_hat = sigmoid(d @ W4 + b4), row-major [128, 4, 784]
                xo = xo_pool.tile([128, 4, D], F32, tag="xo")
                for t in range(4):
                    for nh in range(2):
                        px = pxp.tile([128, NHALF], F32, tag="x")
                        if bias_mm:
                            nc.tensor.matmul(
                                px[:], ones_row[:],
                                b4row[:, nh * NHALF:(nh + 1) * NHALF],
                                start=True, stop=False,
                            )
                        for k in range(2):
                            nc.tensor.matmul(
                                px[:],
                                dT[:, k * 512 + t * 128: k * 512 + t * 128 + 128],
                                W4sb[:, k * D + nh * NHALF: k * D + (nh + 1) * NHALF],
                                start=(k == 0 and not bias_mm), stop=(k == 1),
                            )
                        nc.scalar.activation(
                            xo[:, t, nh * NHALF:(nh + 1) * NHALF], px[:],
                            AF.Sigmoid,
                        )
                nc.sync.dma_start(y_r[s], xo[:])

    nc.finalize()
    return nc


_cache: dict = {}


def _get_nc(rows: int, n_cores: int, bias_mm: bool):
    key = (rows, n_cores, bias_mm)
    if key not in _cache:
        _cache[key] = _build(rows, n_cores, bias_mm)
    return _cache[key]


def kernel(**inputs):
    x = np.ascontiguousarray(np.asarray(inputs["x"], dtype=np.float32))
    rows = x.shape[0]
    n_cores = N_CORES
    rows_pc = rows // n_cores
    bias_mm = not np.allclose(np.asarray(inputs["b4"]), 0.0)
    nc = _get_nc(rows_pc, n_cores, bias_mm)

    w_keys = ["W1", "b1", "W2", "b2", "memory", "W3", "b3", "W4", "b4"]
    weights = {
        k: np.ascontiguousarray(np.asarray(inputs[k], dtype=np.float32))
        for k in w_keys
    }
    in_maps = [
        {"x": x[c * rows_pc:(c + 1) * rows_pc], **weights}
        for c in range(n_cores)
    ]
    res = run_bass_kernel_spmd(
        nc, in_maps, list(range(n_cores)), trace=TRACE
    )
    kernel.last_result = res
    y = np.concatenate([res.results[c]["y"] for c in range(n_cores)], axis=0)
    return y.astype(np.float32)



# revision 9
# speedup vs baseline: 1.3586x; 1.3586x over previous
"""HardAttentionMemoryAE Trainium2 kernel (v2: software-pipelined).

Data-parallel over 8 NeuronCores: x sharded along batch, weights + 50x128
memory bank replicated. Per core the pipeline runs in "transposed
activation" layout (features on partitions, rows on the free dim) so every
matmul contracts along partitions, with a row-major detour for the top-k
masking (per-row ops need rows on partitions).

v2 changes vs v1:
- Emission order software-pipelines slab s's encoder against slab s-1's
  attention/decoder tail so the in-order PE queue never idles (keeps the
  tensor engine p-state at max).
- Top-k thresholding runs on RAW sims (scale-invariant); 1/||z|| is folded
  into the Exp activation's per-partition scale operand.
- Row norms: z row-tiles are transposed on the PE, squared+row-reduced on
  DVE, and 1/sqrt computed with the int-bit-trick + 2 Newton steps on DVE
  (all [128,4] column-layout ops; no serial [1,512] work, no Sqrt table).
- Sigmoid replaced by 0.5*tanh(0.5x)+0.5: tanh/exp/relu/identity/copy all
  live in one activation table set -> zero steady-state ACT_TABLE_LOADs.
  The affine runs on the idle GpSimd(Pool) engine.
- PSUM: 5 rings x {2,2,2,1,1} banks so encoder/decoder/topk phases don't
  serialize on one bank.
"""
import numpy as np
import concourse.bass as bass
import concourse.mybir as mybir
from concourse import bacc
from concourse.tile import TileContext
from concourse.masks import make_identity
from concourse.bass_utils import run_bass_kernel_spmd

F32 = mybir.dt.float32
F32R = mybir.dt.float32r
I32 = mybir.dt.int32
AF = mybir.ActivationFunctionType
ALU = mybir.AluOpType

B_FULL = 65536
D = 784          # input dim
E = 128          # embed dim
M = 50           # memory slots
H = 256          # hidden
N_CORES = 8
SLAB = 512       # rows per slab (4 row-tiles of 128)
NHALF = 392      # final matmul N split (per PSUM bank, >=256 keeps f32r rate)

TRACE = False    # set by test harness for profiling runs

# engine assignment for the 7 per-slab xT PSUM->SBUF copies
XT_COPY_ENG = ["v", "s", "v", "s", "v", "s", "v"]
# engine for the 8 final-layer affine fixups ("p" = gpsimd/pool)
AFF_ENG = ["p"] * 8


def _build(rows: int, n_cores: int, bias_mm: bool):
    nc = bacc.Bacc(
        "TRN2", target_bir_lowering=False, debug=False,
        enable_asserts=True, num_devices=n_cores
    )
    x = nc.dram_tensor("x", [rows, D], F32, kind="ExternalInput")
    W1 = nc.dram_tensor("W1", [D, H], F32, kind="ExternalInput")
    b1 = nc.dram_tensor("b1", [H], F32, kind="ExternalInput")
    W2 = nc.dram_tensor("W2", [H, E], F32, kind="ExternalInput")
    b2 = nc.dram_tensor("b2", [E], F32, kind="ExternalInput")
    mem = nc.dram_tensor("memory", [M, E], F32, kind="ExternalInput")
    W3 = nc.dram_tensor("W3", [E, H], F32, kind="ExternalInput")
    b3 = nc.dram_tensor("b3", [H], F32, kind="ExternalInput")
    W4 = nc.dram_tensor("W4", [H, D], F32, kind="ExternalInput")
    b4 = nc.dram_tensor("b4", [D], F32, kind="ExternalInput")
    y = nc.dram_tensor("y", [rows, D], F32, kind="ExternalOutput")

    n_slabs = rows // SLAB
    # x col chunks for the transpose: 6 aligned chunks + one overlapping
    # final chunk [656, 784) whose first 112 weight rows are zeroed.
    COFF = [0, 128, 256, 384, 512, 640, D - 128]

    x_r = x[:].rearrange("(s t p) c -> s p t c", p=128, t=4)
    y_r = y[:].rearrange("(s t p) c -> s p t c", p=128, t=4)

    with TileContext(nc) as tc:
        with (
            tc.tile_pool(name="const", bufs=1) as cpool,
            tc.tile_pool(name="xr", bufs=2) as xr_pool,
            tc.tile_pool(name="xT", bufs=2) as xT_pool,
            tc.tile_pool(name="hT", bufs=2) as hT_pool,
            tc.tile_pool(name="zT", bufs=2) as zT_pool,
            tc.tile_pool(name="small", bufs=2) as sm_pool,
            tc.tile_pool(name="xout", bufs=2) as xo_pool,
            tc.tile_pool(name="pbig", bufs=2, space="PSUM") as pbig,
            tc.tile_pool(name="pmid", bufs=2, space="PSUM") as pmid,
            tc.tile_pool(name="pxp", bufs=2, space="PSUM") as pxp,
            tc.tile_pool(name="penc", bufs=1, space="PSUM") as penc,
            tc.tile_pool(name="pmisc", bufs=1, space="PSUM") as pmisc,
        ):
            # ---------------- one-time setup ----------------
            W1sb = cpool.tile([128, 7 * H], F32R)
            zpad = cpool.tile([128, H], F32)
            nc.vector.memset(zpad[:], 0.0)
            nc.scalar.copy(W1sb[:, 6 * H:], zpad[:])
            for c in range(6):
                nc.gpsimd.dma_start(
                    W1sb[:, c * H:(c + 1) * H], W1[COFF[c]:COFF[c] + 128, :]
                )
            nc.gpsimd.dma_start(W1sb[112:128, 6 * H:7 * H], W1[768:D, :])
            W2sb = cpool.tile([128, 2 * E], F32R)
            for m in range(2):
                nc.gpsimd.dma_start(
                    W2sb[:, m * E:(m + 1) * E], W2[m * 128:(m + 1) * 128, :]
                )
            W3sb = cpool.tile([128, H], F32R)
            nc.gpsimd.dma_start(W3sb[:], W3[:])
            W4sb = cpool.tile([128, 2 * D], F32R)
            for k in range(2):
                nc.gpsimd.dma_start(
                    W4sb[:, k * D:(k + 1) * D], W4[k * 128:(k + 1) * 128, :]
                )
            memsb = cpool.tile([M, E], F32R)
            nc.gpsimd.dma_start(memsb[:], mem[:])

            b1sb = cpool.tile([128, 2], F32)
            nc.sync.dma_start(b1sb[:], b1[:].rearrange("(m p) -> p m", p=128))
            b2sb = cpool.tile([128, 1], F32)
            nc.sync.dma_start(b2sb[:], b2[:].rearrange("(p o) -> p o", o=1))
            b3sb = cpool.tile([128, 2], F32)
            nc.sync.dma_start(b3sb[:], b3[:].rearrange("(m p) -> p m", p=128))
            if bias_mm:
                b4row = cpool.tile([1, D], F32R)
                nc.gpsimd.dma_start(b4row[:], b4[:].rearrange("(o c) -> o c", o=1))
                ones_row = cpool.tile([1, 128], F32R)
                onesr_f = cpool.tile([1, 128], F32)
                nc.vector.memset(onesr_f[:], 1.0)
                nc.scalar.copy(ones_row[:], onesr_f[:])

            ident_f = cpool.tile([128, 128], F32)
            make_identity(nc, ident_f[:])
            ident = cpool.tile([128, 128], F32R)
            nc.scalar.copy(ident[:], ident_f[:])

            # normalized memory, transposed: mem_normT [E, M]
            memf = cpool.tile([M, E], F32)
            nc.sync.dma_start(memf[:], mem[:])
            msq = cpool.tile([M, E], F32)
            nc.scalar.square(msq[:], memf[:])
            mss = cpool.tile([M, 1], F32)
            nc.vector.tensor_reduce(mss[:], msq[:], mybir.AxisListType.X, ALU.add)
            nc.scalar.sqrt(mss[:], mss[:])
            nc.vector.tensor_scalar_max(mss[:], mss[:], 1e-12)
            minv = cpool.tile([M, 1], F32)
            nc.vector.reciprocal(minv[:], mss[:])
            mnorm = cpool.tile([M, E], F32R)
            nc.vector.tensor_scalar_mul(mnorm[:], memf[:], minv[:, 0:1])
            p_mn = pmisc.tile([128, 512], F32, tag="misc", name="p_mn")
            nc.tensor.transpose(p_mn[:E, :M].bitcast(F32R), mnorm[:], ident[:M, :M])
            mnT = cpool.tile([E, M], F32R)
            nc.scalar.copy(mnT[:], p_mn[:E, :M])

            # ---------------- per-slab stage emitters ----------------
            st = {}   # slab index -> dict of live tiles

            def copy_eng(which):
                return {"v": nc.vector, "s": nc.scalar, "p": nc.gpsimd}[which]

            def emit_dma_in(s):
                d = st.setdefault(s, {})
                d["xr"] = xr_pool.tile([128, 4, D], F32R, tag="xr",
                                       name=f"xr_{s}")
                nc.gpsimd.dma_start(d["xr"][:], x_r[s])

            def emit_transp(s, chunks):
                d = st[s]
                xT = d.setdefault("xT", {})
                for c in chunks:
                    xT[c] = xT_pool.tile([128, SLAB], F32R, tag=f"xt{c}",
                                         name=f"xt{c}_{s}")
                    ptr = pmid.tile([128, 512], F32, tag="mid",
                                    name=f"ptr{c}_{s}")
                    for t in range(4):
                        nc.tensor.transpose(
                            ptr[:, t * 128:(t + 1) * 128].bitcast(F32R),
                            d["xr"][:, t, COFF[c]:COFF[c] + 128],
                            ident[:],
                        )
                    eng = copy_eng(XT_COPY_ENG[c])
                    if XT_COPY_ENG[c] == "s":
                        nc.scalar.copy(xT[c][:], ptr[:])
                    else:
                        eng.tensor_copy(xT[c][:], ptr[:])

            def emit_p1(s, m):
                d = st[s]
                if "hT" not in d:
                    d["hT"] = hT_pool.tile([128, 1024], F32R, tag="hT",
                                           name=f"hT_{s}")
                ph = pbig.tile([128, 512], F32, tag="big", name=f"ph{m}_{s}")
                for c in range(7):
                    nc.tensor.matmul(
                        ph[:],
                        W1sb[:, c * H + m * 128: c * H + m * 128 + 128],
                        d["xT"][c][:],
                        start=(c == 0), stop=(c == 6),
                    )
                nc.scalar.activation(
                    d["hT"][:, m * 512:(m + 1) * 512], ph[:],
                    AF.Relu, bias=b1sb[:, m:m + 1],
                )

            def emit_p2(s):
                d = st[s]
                pz = penc.tile([128, 512], F32, tag="enc", name=f"pz_{s}")
                for m in range(2):
                    nc.tensor.matmul(
                        pz[:], W2sb[:, m * E:(m + 1) * E],
                        d["hT"][:, m * 512:(m + 1) * 512],
                        start=(m == 0), stop=(m == 1),
                    )
                d["zT"] = zT_pool.tile([128, SLAB], F32R, tag="zT",
                                       name=f"zT_{s}")
                nc.scalar.activation(d["zT"][:], pz[:], AF.Identity,
                                     bias=b2sb[:, 0:1])

            def emit_norm(s):
                # row norms in column layout: transpose z row-tiles on PE,
                # square+reduce on DVE, rsqrt via bit trick + 2 Newton steps.
                d = st[s]
                zrm = pmisc.tile([128, 512], F32, tag="misc", name=f"zrm_{s}")
                for t in range(4):
                    nc.tensor.transpose(
                        zrm[:, t * 128:(t + 1) * 128].bitcast(F32R),
                        d["zT"][:, t * 128:(t + 1) * 128],
                        ident[:],
                    )
                zsqc = sm_pool.tile([128, 512], F32, tag="zsqc",
                                    name=f"zsqc_{s}")
                nc.scalar.square(zsqc[:], zrm[:])
                nsq = sm_pool.tile([128, 4], F32, tag="nsq", name=f"nsq_{s}")
                nc.vector.tensor_reduce(
                    nsq[:], zsqc[:].rearrange("p (t c) -> p t c", c=128),
                    mybir.AxisListType.X, ALU.add,
                )
                # inv = 1/sqrt(nsq): magic-constant seed + 2 Newton steps
                seed_i = sm_pool.tile([128, 4], I32, tag="seed_i",
                                      name=f"seed_i_{s}")
                nc.vector.tensor_scalar(
                    out=seed_i[:], in0=nsq[:].bitcast(I32),
                    scalar1=1, scalar2=None, op0=ALU.logical_shift_right,
                )
                y0_i = sm_pool.tile([128, 4], I32, tag="y0_i",
                                    name=f"y0_i_{s}")
                nc.vector.tensor_scalar(
                    out=y0_i[:], in0=seed_i[:],
                    scalar1=-1, scalar2=0x5F3759DF, op0=ALU.mult,
                    op1=ALU.add,
                )
                # y0_i = 0x5f3759df - (bits(nsq) >> 1): rsqrt seed
                h = sm_pool.tile([128, 4], F32, tag="h", name=f"h_{s}")
                nc.vector.tensor_scalar(
                    out=h[:], in0=nsq[:], scalar1=0.5, scalar2=1e-30,
                    op0=ALU.mult, op1=ALU.max,
                )
                ycur = y0_i[:].bitcast(F32)
                for it in range(2):
                    a = sm_pool.tile([128, 4], F32, tag=f"nta{it}",
                                     name=f"nta{it}_{s}")
                    nc.vector.tensor_tensor(a[:], ycur, ycur, ALU.mult)
                    b_ = sm_pool.tile([128, 4], F32, tag=f"ntb{it}",
                                      name=f"ntb{it}_{s}")
                    nc.vector.tensor_tensor(b_[:], a[:], h[:], ALU.mult)
                    c_ = sm_pool.tile([128, 4], F32, tag=f"ntc{it}",
                                      name=f"ntc{it}_{s}")
                    nc.vector.tensor_scalar(
                        out=c_[:], in0=b_[:], scalar1=-1.0, scalar2=1.5,
                        op0=ALU.mult, op1=ALU.add,
                    )
                    ynext = sm_pool.tile([128, 4], F32, tag=f"nty{it}",
                                         name=f"nty{it}_{s}")
                    nc.vector.tensor_tensor(ynext[:], ycur, c_[:], ALU.mult)
                    ycur = ynext[:]
                d["invcol"] = ycur

            def emit_sims(s):
                d = st[s]
                psim = penc.tile([128, 512], F32, tag="enc", name=f"psim_{s}")
                for t in range(4):
                    nc.tensor.matmul(
                        psim[:, t * M:(t + 1) * M],
                        d["zT"][:, t * 128:(t + 1) * 128], mnT[:],
                        start=True, stop=True,
                    )
                d["psim"] = psim

            def emit_topk(s):
                d = st[s]
                simsb = sm_pool.tile([128, 4 * M], F32, tag="simsb",
                                     name=f"simsb_{s}")
                nc.vector.tensor_copy(simsb[:], d["psim"][:, :4 * M])
                m8 = sm_pool.tile([128, 32], F32, tag="m8", name=f"m8_{s}")
                msk = sm_pool.tile([128, 4 * M], F32, tag="msk",
                                   name=f"msk_{s}")
                pexp = sm_pool.tile([128, 4 * M], F32, tag="pexp",
                                    name=f"pexp_{s}")
                den = sm_pool.tile([128, 4], F32, tag="den", name=f"den_{s}")
                for t in range(4):
                    nc.vector.max(m8[:, t * 8:(t + 1) * 8],
                                  simsb[:, t * M:(t + 1) * M])
                    nc.vector.scalar_tensor_tensor(
                        out=msk[:, t * M:(t + 1) * M],
                        in0=simsb[:, t * M:(t + 1) * M],
                        scalar=m8[:, t * 8 + 4:t * 8 + 5],
                        in1=simsb[:, t * M:(t + 1) * M],
                        op0=ALU.is_ge, op1=ALU.mult,
                    )
                    nc.scalar.activation(
                        pexp[:, t * M:(t + 1) * M], msk[:, t * M:(t + 1) * M],
                        AF.Exp, scale=d["invcol"][:, t:t + 1],
                        accum_out=den[:, t:t + 1],
                    )
                rden = sm_pool.tile([128, 4], F32, tag="rden",
                                    name=f"rden_{s}")
                nc.vector.reciprocal(rden[:], den[:])
                attn = sm_pool.tile([128, 4 * M], F32R, tag="attn",
                                    name=f"attn_{s}")
                for t in range(4):
                    nc.gpsimd.tensor_scalar_mul(
                        attn[:, t * M:(t + 1) * M],
                        pexp[:, t * M:(t + 1) * M],
                        rden[:, t:t + 1],
                    )
                d["attn"] = attn

            def emit_pat(s):
                d = st[s]
                pat = pmisc.tile([128, 512], F32, tag="misc", name=f"pat_{s}")
                for t in range(4):
                    nc.tensor.transpose(
                        pat[:M, t * 128:(t + 1) * 128].bitcast(F32R),
                        d["attn"][:, t * M:(t + 1) * M], ident[:],
                    )
                attnT = sm_pool.tile([M, SLAB], F32R, tag="attnT",
                                     name=f"attnT_{s}")
                nc.vector.tensor_copy(attnT[:], pat[:M, :])
                d["attnT"] = attnT

            def emit_zmem(s):
                d = st[s]
                pzm = pmisc.tile([128, 512], F32, tag="misc", name=f"pzm_{s}")
                nc.tensor.matmul(pzm[:], memsb[:], d["attnT"][:],
                                 start=True, stop=True)
                zm = sm_pool.tile([128, SLAB], F32R, tag="zm", name=f"zm_{s}")
                nc.scalar.copy(zm[:], pzm[:])
                d["zm"] = zm

            def emit_dec(s):
                d = st[s]
                d["dT"] = hT_pool.tile([128, 1024], F32R, tag="dT",
                                       name=f"dT_{s}")
                for m in range(2):
                    pd = pbig.tile([128, 512], F32, tag="big",
                                   name=f"pd{m}_{s}")
                    nc.tensor.matmul(
                        pd[:], W3sb[:, m * 128:(m + 1) * 128], d["zm"][:],
                        start=True, stop=True,
                    )
                    nc.vector.tensor_scalar(
                        out=d["dT"][:, m * 512:(m + 1) * 512], in0=pd[:],
                        scalar1=b3sb[:, m:m + 1], scalar2=0.0,
                        op0=ALU.add, op1=ALU.max,
                    )

            def emit_final(s, tiles):
                d = st[s]
                if "xo" not in d:
                    d["xo"] = xo_pool.tile([128, 4, D], F32, tag="xo",
                                           name=f"xo_{s}")
                for t in tiles:
                    for nh in range(2):
                        px = pxp.tile([128, NHALF], F32, tag="x",
                                      name=f"px{t}{nh}_{s}")
                        if bias_mm:
                            nc.tensor.matmul(
                                px[:], ones_row[:],
                                b4row[:, nh * NHALF:(nh + 1) * NHALF],
                                start=True, stop=False,
                            )
                        for k in range(2):
                            nc.tensor.matmul(
                                px[:],
                                d["dT"][:, k * 512 + t * 128:
                                        k * 512 + t * 128 + 128],
                                W4sb[:, k * D + nh * NHALF:
                                     k * D + (nh + 1) * NHALF],
                                start=(k == 0 and not bias_mm), stop=(k == 1),
                            )
                        xs = d["xo"][:, t, nh * NHALF:(nh + 1) * NHALF]
                        nc.scalar.activation(xs, px[:], AF.Tanh, scale=0.5)
                        aff = copy_eng(AFF_ENG[t * 2 + nh])
                        aff.tensor_scalar(
                            out=xs, in0=xs, scalar1=0.5, scalar2=0.5,
                            op0=ALU.mult, op1=ALU.add,
                        )

            def emit_out(s):
                nc.sync.dma_start(y_r[s], st[s]["xo"][:])
                # drop references to this slab's tiles
                del st[s]

            # ---------------- software-pipelined slab loop ----------------
            emit_dma_in(0)
            for s in range(n_slabs):
                if s + 1 < n_slabs:
                    emit_dma_in(s + 1)
                emit_transp(s, [0, 1, 2, 3, 4])
                if s > 0:
                    emit_pat(s - 1)
                emit_transp(s, [5, 6])
                if s > 0:
                    emit_zmem(s - 1)
                emit_p1(s, 0)
                if s > 0:
                    emit_dec(s - 1)
                emit_p1(s, 1)
                if s > 0:
                    emit_final(s - 1, [0, 1])
                emit_p2(s)
                if s > 0:
                    emit_final(s - 1, [2, 3])
                    emit_out(s - 1)
                emit_norm(s)
                emit_sims(s)
                emit_topk(s)
            s = n_slabs - 1
            emit_pat(s)
            emit_zmem(s)
            emit_dec(s)
            emit_final(s, [0, 1, 2, 3])
            emit_out(s)

    nc.finalize()
    return nc


_cache: dict = {}


def _get_nc(rows: int, n_cores: int, bias_mm: bool):
    key = (rows, n_cores, bias_mm)
    if key not in _cache:
        _cache[key] = _build(rows, n_cores, bias_mm)
    return _cache[key]


def kernel(**inputs):
    x = np.ascontiguousarray(np.asarray(inputs["x"], dtype=np.float32))
    rows = x.shape[0]
    n_cores = N_CORES
    rows_pc = rows // n_cores
    bias_mm = not np.allclose(np.asarray(inputs["b4"]), 0.0)
    nc = _get_nc(rows_pc, n_cores, bias_mm)

    w_keys = ["W1", "b1", "W2", "b2", "memory", "W3", "b3", "W4", "b4"]
    weights = {
        k: np.ascontiguousarray(np.asarray(inputs[k], dtype=np.float32))
        for k in w_keys
    }
    in_maps = [
        {"x": x[c * rows_pc:(c + 1) * rows_pc], **weights}
        for c in range(n_cores)
    ]
    res = run_bass_kernel_spmd(
        nc, in_maps, list(range(n_cores)), trace=TRACE
    )
    kernel.last_result = res
    y = np.concatenate([res.results[c]["y"] for c in range(n_cores)], axis=0)
    return y.astype(np.float32)


# revision 19
# speedup vs baseline: 1.5272x; 1.1241x over previous
"""HardAttentionMemoryAE Trainium2 kernel (v2: software-pipelined).

Data-parallel over 8 NeuronCores: x sharded along batch, weights + 50x128
memory bank replicated. Per core the pipeline runs in "transposed
activation" layout (features on partitions, rows on the free dim) so every
matmul contracts along partitions, with a row-major detour for the top-k
masking (per-row ops need rows on partitions).

v2 changes vs v1:
- Emission order software-pipelines slab s's encoder against slab s-1's
  attention/decoder tail so the in-order PE queue never idles (keeps the
  tensor engine p-state at max).
- Top-k thresholding runs on RAW sims (scale-invariant); 1/||z|| is folded
  into the Exp activation's per-partition scale operand.
- Row norms: z row-tiles are transposed on the PE, squared+row-reduced on
  DVE, and 1/sqrt computed with the int-bit-trick + 2 Newton steps on DVE
  (all [128,4] column-layout ops; no serial [1,512] work, no Sqrt table).
- Sigmoid replaced by 0.5*tanh(0.5x)+0.5: tanh/exp/relu/identity/copy all
  live in one activation table set -> zero steady-state ACT_TABLE_LOADs.
  The affine runs on the idle GpSimd(Pool) engine.
- PSUM: 5 rings x {2,2,2,1,1} banks so encoder/decoder/topk phases don't
  serialize on one bank.
"""
import numpy as np
import concourse.bass as bass
import concourse.mybir as mybir
from concourse import bacc
from concourse.tile import TileContext
from concourse.masks import make_identity
from concourse.bass_utils import run_bass_kernel_spmd

F32 = mybir.dt.float32
F32R = mybir.dt.float32r
I32 = mybir.dt.int32
AF = mybir.ActivationFunctionType
ALU = mybir.AluOpType

B_FULL = 65536
D = 784          # input dim
E = 128          # embed dim
M = 50           # memory slots
H = 256          # hidden
N_CORES = 8
SLAB = 512       # rows per slab (4 row-tiles of 128)
NHALF = 392      # final matmul N split (per PSUM bank, >=256 keeps f32r rate)

TRACE = False    # set by test harness for profiling runs

# engine assignment for the 7 per-slab xT PSUM->SBUF copies
XT_COPY_ENG = ["v", "s", "v", "v", "s", "v", "v"]
NEWTON_STEPS = 1


def _build(rows: int, n_cores: int, bias_mm: bool):
    nc = bacc.Bacc(
        "TRN2", target_bir_lowering=False, debug=False,
        enable_asserts=True, num_devices=n_cores
    )
    x = nc.dram_tensor("x", [rows, D], F32, kind="ExternalInput")
    W1 = nc.dram_tensor("W1", [D, H], F32, kind="ExternalInput")
    b1 = nc.dram_tensor("b1", [H], F32, kind="ExternalInput")
    W2 = nc.dram_tensor("W2", [H, E], F32, kind="ExternalInput")
    b2 = nc.dram_tensor("b2", [E], F32, kind="ExternalInput")
    mem = nc.dram_tensor("memory", [M, E], F32, kind="ExternalInput")
    W3 = nc.dram_tensor("W3", [E, H], F32, kind="ExternalInput")
    b3 = nc.dram_tensor("b3", [H], F32, kind="ExternalInput")
    W4 = nc.dram_tensor("W4", [H, D], F32, kind="ExternalInput")
    b4 = nc.dram_tensor("b4", [D], F32, kind="ExternalInput")
    y = nc.dram_tensor("y", [rows, D], F32, kind="ExternalOutput")

    n_slabs = rows // SLAB
    # x col chunks for the transpose: 6 aligned chunks + one overlapping
    # final chunk [656, 784) whose first 112 weight rows are zeroed.
    COFF = [0, 128, 256, 384, 512, 640, D - 128]

    x_r = x[:].rearrange("(s t p) c -> s p t c", p=128, t=4)
    y_r = y[:].rearrange("(s t p) c -> s p t c", p=128, t=4)

    with TileContext(nc) as tc:
        with (
            tc.tile_pool(name="const", bufs=1) as cpool,
            tc.tile_pool(name="xr", bufs=2) as xr_pool,
            tc.tile_pool(name="xT", bufs=2) as xT_pool,
            tc.tile_pool(name="hT", bufs=2) as hT_pool,
            tc.tile_pool(name="zT", bufs=2) as zT_pool,
            tc.tile_pool(name="small", bufs=2) as sm_pool,
            tc.tile_pool(name="xout", bufs=2) as xo_pool,
            tc.tile_pool(name="pbig", bufs=2, space="PSUM") as pbig,
            tc.tile_pool(name="pmid", bufs=2, space="PSUM") as pmid,
            tc.tile_pool(name="pxp", bufs=1, space="PSUM") as pxp,
            tc.tile_pool(name="penc", bufs=1, space="PSUM") as penc,
            tc.tile_pool(name="pmisc", bufs=1, space="PSUM") as pmisc,
        ):
            # ---------------- one-time setup ----------------
            W1sb = cpool.tile([128, 7 * H], F32R)
            zpad = cpool.tile([128, H], F32)
            nc.vector.memset(zpad[:], 0.0)
            nc.scalar.copy(W1sb[:, 6 * H:], zpad[:])
            for c in range(6):
                nc.gpsimd.dma_start(
                    W1sb[:, c * H:(c + 1) * H], W1[COFF[c]:COFF[c] + 128, :]
                )
            nc.gpsimd.dma_start(W1sb[112:128, 6 * H:7 * H], W1[768:D, :])
            W2sb = cpool.tile([128, 2 * E], F32R)
            for m in range(2):
                nc.gpsimd.dma_start(
                    W2sb[:, m * E:(m + 1) * E], W2[m * 128:(m + 1) * 128, :]
                )
            W3sb = cpool.tile([128, H], F32R)
            nc.gpsimd.dma_start(W3sb[:], W3[:])
            W4sb = cpool.tile([128, 2 * D], F32R)
            for k in range(2):
                nc.gpsimd.dma_start(
                    W4sb[:, k * D:(k + 1) * D], W4[k * 128:(k + 1) * 128, :]
                )
            b1sb = cpool.tile([128, 2], F32)
            nc.sync.dma_start(b1sb[:], b1[:].rearrange("(m p) -> p m", p=128))
            b2sb = cpool.tile([128, 1], F32)
            nc.sync.dma_start(b2sb[:], b2[:].rearrange("(p o) -> p o", o=1))
            b3sb = cpool.tile([128, 2], F32)
            nc.sync.dma_start(b3sb[:], b3[:].rearrange("(m p) -> p m", p=128))
            if bias_mm:
                b4row = cpool.tile([1, D], F32R)
                nc.gpsimd.dma_start(b4row[:], b4[:].rearrange("(o c) -> o c", o=1))
                ones_row = cpool.tile([1, 128], F32R)
                onesr_f = cpool.tile([1, 128], F32)
                nc.vector.memset(onesr_f[:], 1.0)
                nc.scalar.copy(ones_row[:], onesr_f[:])

            ident_f = cpool.tile([128, 128], F32)
            make_identity(nc, ident_f[:])
            ident = cpool.tile([128, 128], F32R)
            nc.scalar.copy(ident[:], ident_f[:])

            # normalized memory, transposed: mem_normT [E, M]
            memf = cpool.tile([M, E], F32)
            nc.sync.dma_start(memf[:], mem[:])
            msq = cpool.tile([M, E], F32)
            nc.scalar.square(msq[:], memf[:])
            mss = cpool.tile([M, 1], F32)
            nc.vector.tensor_reduce(mss[:], msq[:], mybir.AxisListType.X, ALU.add)
            nc.scalar.sqrt(mss[:], mss[:])
            nc.vector.tensor_scalar_max(mss[:], mss[:], 1e-12)
            minv = cpool.tile([M, 1], F32)
            nc.vector.reciprocal(minv[:], mss[:])
            mnorm = cpool.tile([M, E], F32R)
            nc.vector.tensor_scalar_mul(mnorm[:], memf[:], minv[:, 0:1])
            p_mn = pmisc.tile([128, 512], F32, tag="misc", name="p_mn")
            nc.tensor.transpose(p_mn[:E, :M].bitcast(F32R), mnorm[:], ident[:M, :M])
            mnT = cpool.tile([E, M], F32R)
            nc.scalar.copy(mnT[:], p_mn[:E, :M])

            # W3m = memory @ W3  [M, H]; decoder contracts attnT against it
            # directly (z_mem never materializes: attn@(mem@W3) == (attn@mem)@W3)
            p_mt = pmisc.tile([128, 512], F32, tag="misc", name="p_mt")
            memr = cpool.tile([M, E], F32R)
            nc.scalar.copy(memr[:], memf[:])
            nc.tensor.transpose(p_mt[:E, :M].bitcast(F32R), memr[:], ident[:M, :M])
            memT = cpool.tile([E, M], F32R)
            nc.scalar.copy(memT[:], p_mt[:E, :M])
            p_w3m = pmisc.tile([128, 512], F32, tag="misc", name="p_w3m")
            nc.tensor.matmul(p_w3m[:M, :H], memT[:], W3sb[:],
                             start=True, stop=True)
            W3msb = cpool.tile([M, H], F32R)
            nc.scalar.copy(W3msb[:], p_w3m[:M, :H])

            # ---------------- per-slab stage emitters ----------------
            st = {}   # slab index -> dict of live tiles

            def copy_eng(which):
                return {"v": nc.vector, "s": nc.scalar, "p": nc.gpsimd}[which]

            def emit_dma_in(s):
                d = st.setdefault(s, {})
                d["xr"] = xr_pool.tile([128, 4, D], F32R, tag="xr",
                                       name=f"xr_{s}")
                nc.gpsimd.dma_start(d["xr"][:], x_r[s])

            def emit_transp(s, chunks):
                d = st[s]
                xT = d.setdefault("xT", {})
                for c in chunks:
                    xT[c] = xT_pool.tile([128, SLAB], F32R, tag=f"xt{c}",
                                         name=f"xt{c}_{s}")
                    ptr = pmid.tile([128, 512], F32, tag="mid",
                                    name=f"ptr{c}_{s}")
                    for t in range(4):
                        nc.tensor.transpose(
                            ptr[:, t * 128:(t + 1) * 128].bitcast(F32R),
                            d["xr"][:, t, COFF[c]:COFF[c] + 128],
                            ident[:],
                        )
                    eng = copy_eng(XT_COPY_ENG[c])
                    if XT_COPY_ENG[c] == "s":
                        nc.scalar.copy(xT[c][:], ptr[:])
                    else:
                        eng.tensor_copy(xT[c][:], ptr[:])

            def emit_p1(s, m):
                d = st[s]
                if "hT" not in d:
                    d["hT"] = hT_pool.tile([128, 1024], F32R, tag="hT",
                                           name=f"hT_{s}")
                ph = pbig.tile([128, 512], F32, tag="big", name=f"ph{m}_{s}")
                for c in range(7):
                    nc.tensor.matmul(
                        ph[:],
                        W1sb[:, c * H + m * 128: c * H + m * 128 + 128],
                        d["xT"][c][:],
                        start=(c == 0), stop=(c == 6),
                    )
                nc.scalar.activation(
                    d["hT"][:, m * 512:(m + 1) * 512], ph[:],
                    AF.Relu, bias=b1sb[:, m:m + 1],
                )

            def emit_p2(s):
                d = st[s]
                pz = penc.tile([128, 512], F32, tag="enc", name=f"pz_{s}")
                for m in range(2):
                    nc.tensor.matmul(
                        pz[:], W2sb[:, m * E:(m + 1) * E],
                        d["hT"][:, m * 512:(m + 1) * 512],
                        start=(m == 0), stop=(m == 1),
                    )
                d["zT"] = zT_pool.tile([128, SLAB], F32R, tag="zT",
                                       name=f"zT_{s}")
                nc.scalar.activation(d["zT"][:], pz[:], AF.Identity,
                                     bias=b2sb[:, 0:1])

            def emit_norm(s):
                # row norms in column layout: transpose z row-tiles on PE,
                # square+reduce on DVE, rsqrt via bit trick + 2 Newton steps.
                d = st[s]
                zrm = pmisc.tile([128, 512], F32, tag="misc", name=f"zrm_{s}")
                for t in range(4):
                    nc.tensor.transpose(
                        zrm[:, t * 128:(t + 1) * 128].bitcast(F32R),
                        d["zT"][:, t * 128:(t + 1) * 128],
                        ident[:],
                    )
                zsqc = sm_pool.tile([128, 512], F32, tag="zsqc",
                                    name=f"zsqc_{s}")
                nc.scalar.square(zsqc[:], zrm[:])
                nsq = sm_pool.tile([128, 4], F32, tag="nsq", name=f"nsq_{s}")
                nc.vector.tensor_reduce(
                    nsq[:], zsqc[:].rearrange("p (t c) -> p t c", c=128),
                    mybir.AxisListType.X, ALU.add,
                )
                # inv = 1/sqrt(nsq): magic-constant seed + 2 Newton steps
                seed_i = sm_pool.tile([128, 4], I32, tag="seed_i",
                                      name=f"seed_i_{s}")
                nc.vector.tensor_scalar(
                    out=seed_i[:], in0=nsq[:].bitcast(I32),
                    scalar1=1, scalar2=None, op0=ALU.logical_shift_right,
                )
                y0_i = sm_pool.tile([128, 4], I32, tag="y0_i",
                                    name=f"y0_i_{s}")
                nc.vector.tensor_scalar(
                    out=y0_i[:], in0=seed_i[:],
                    scalar1=-1, scalar2=0x5F3759DF, op0=ALU.mult,
                    op1=ALU.add,
                )
                # y0_i = 0x5f3759df - (bits(nsq) >> 1): rsqrt seed
                h = sm_pool.tile([128, 4], F32, tag="h", name=f"h_{s}")
                nc.vector.tensor_scalar(
                    out=h[:], in0=nsq[:], scalar1=0.5, scalar2=1e-30,
                    op0=ALU.mult, op1=ALU.max,
                )
                ycur = y0_i[:].bitcast(F32)
                for it in range(NEWTON_STEPS):
                    a = sm_pool.tile([128, 4], F32, tag=f"nta{it}",
                                     name=f"nta{it}_{s}")
                    nc.vector.tensor_tensor(a[:], ycur, ycur, ALU.mult)
                    b_ = sm_pool.tile([128, 4], F32, tag=f"ntb{it}",
                                      name=f"ntb{it}_{s}")
                    nc.vector.tensor_tensor(b_[:], a[:], h[:], ALU.mult)
                    c_ = sm_pool.tile([128, 4], F32, tag=f"ntc{it}",
                                      name=f"ntc{it}_{s}")
                    nc.vector.tensor_scalar(
                        out=c_[:], in0=b_[:], scalar1=-1.0, scalar2=1.5,
                        op0=ALU.mult, op1=ALU.add,
                    )
                    ynext = sm_pool.tile([128, 4], F32, tag=f"nty{it}",
                                         name=f"nty{it}_{s}")
                    nc.vector.tensor_tensor(ynext[:], ycur, c_[:], ALU.mult)
                    ycur = ynext[:]
                d["invcol"] = ycur

            def emit_sims(s):
                d = st[s]
                psim = penc.tile([128, 512], F32, tag="enc", name=f"psim_{s}")
                for t in range(4):
                    nc.tensor.matmul(
                        psim[:, t * M:(t + 1) * M],
                        d["zT"][:, t * 128:(t + 1) * 128], mnT[:],
                        start=True, stop=True,
                    )
                d["psim"] = psim

            def emit_topk(s):
                d = st[s]
                simsb = sm_pool.tile([128, 4 * M], F32, tag="simsb",
                                     name=f"simsb_{s}")
                nc.vector.tensor_copy(simsb[:], d["psim"][:, :4 * M])
                m8 = sm_pool.tile([128, 32], F32, tag="m8", name=f"m8_{s}")
                msk = sm_pool.tile([128, 4 * M], F32, tag="msk",
                                   name=f"msk_{s}")
                pexp = sm_pool.tile([128, 4 * M], F32, tag="pexp",
                                    name=f"pexp_{s}")
                den = sm_pool.tile([128, 4], F32, tag="den", name=f"den_{s}")
                for t in range(4):
                    nc.vector.max(m8[:, t * 8:(t + 1) * 8],
                                  simsb[:, t * M:(t + 1) * M])
                    nc.vector.scalar_tensor_tensor(
                        out=msk[:, t * M:(t + 1) * M],
                        in0=simsb[:, t * M:(t + 1) * M],
                        scalar=m8[:, t * 8 + 4:t * 8 + 5],
                        in1=simsb[:, t * M:(t + 1) * M],
                        op0=ALU.is_ge, op1=ALU.mult,
                    )
                    nc.scalar.activation(
                        pexp[:, t * M:(t + 1) * M], msk[:, t * M:(t + 1) * M],
                        AF.Exp, scale=d["invcol"][:, t:t + 1],
                        accum_out=den[:, t:t + 1],
                    )
                rden = sm_pool.tile([128, 4], F32, tag="rden",
                                    name=f"rden_{s}")
                nc.vector.reciprocal(rden[:], den[:])
                attn = sm_pool.tile([128, 4 * M], F32R, tag="attn",
                                    name=f"attn_{s}")
                for t in range(4):
                    nc.vector.tensor_scalar_mul(
                        attn[:, t * M:(t + 1) * M],
                        pexp[:, t * M:(t + 1) * M],
                        rden[:, t:t + 1],
                    )
                d["attn"] = attn

            def emit_pat(s):
                d = st[s]
                pat = pmisc.tile([128, 512], F32, tag="misc", name=f"pat_{s}")
                for t in range(4):
                    nc.tensor.transpose(
                        pat[:M, t * 128:(t + 1) * 128].bitcast(F32R),
                        d["attn"][:, t * M:(t + 1) * M], ident[:],
                    )
                attnT = sm_pool.tile([M, SLAB], F32R, tag="attnT",
                                     name=f"attnT_{s}")
                nc.vector.tensor_copy(attnT[:], pat[:M, :])
                d["attnT"] = attnT

            def emit_dec(s):
                d = st[s]
                d["dT"] = hT_pool.tile([128, 1024], F32R, tag="dT",
                                       name=f"dT_{s}")
                for m in range(2):
                    pd = pbig.tile([128, 512], F32, tag="big",
                                   name=f"pd{m}_{s}")
                    nc.tensor.matmul(
                        pd[:], W3msb[:, m * 128:(m + 1) * 128], d["attnT"][:],
                        start=True, stop=True,
                    )
                    nc.vector.tensor_scalar(
                        out=d["dT"][:, m * 512:(m + 1) * 512], in0=pd[:],
                        scalar1=b3sb[:, m:m + 1], scalar2=0.0,
                        op0=ALU.add, op1=ALU.max,
                    )

            def emit_final(s, tiles):
                d = st[s]
                if "xo" not in d:
                    d["xo"] = xo_pool.tile([128, 4, D], F32, tag="xo",
                                           name=f"xo_{s}")
                for t in tiles:
                    px = pxp.tile([128, 1024], F32, tag="x",
                                  name=f"px{t}_{s}")
                    for nh in range(2):
                        pxh = px[:, nh * 512:nh * 512 + NHALF]
                        if bias_mm:
                            nc.tensor.matmul(
                                pxh, ones_row[:],
                                b4row[:, nh * NHALF:(nh + 1) * NHALF],
                                start=True, stop=False,
                            )
                        for k in range(2):
                            nc.tensor.matmul(
                                pxh,
                                d["dT"][:, k * 512 + t * 128:
                                        k * 512 + t * 128 + 128],
                                W4sb[:, k * D + nh * NHALF:
                                     k * D + (nh + 1) * NHALF],
                                start=(k == 0 and not bias_mm), stop=(k == 1),
                            )
                    # sigmoid(v) = 0.5*tanh(0.5 v)+0.5; one 3D-AP tanh per
                    # row tile, affine fixup on the (otherwise idle) pool
                    pxv = px[:].rearrange("p (n c) -> p n c", n=2)[:, :, :NHALF]
                    xov = d["xo"][:, t, :].rearrange("p (n c) -> p n c", n=2)
                    nc.scalar.activation(xov, pxv, AF.Tanh, scale=0.5)
                    nc.gpsimd.tensor_scalar(
                        out=d["xo"][:, t, :], in0=d["xo"][:, t, :],
                        scalar1=0.5, scalar2=0.5,
                        op0=ALU.mult, op1=ALU.add,
                    )

            def emit_out(s):
                nc.sync.dma_start(y_r[s], st[s]["xo"][:])
                # drop references to this slab's tiles
                del st[s]

            # ---------------- software-pipelined slab loop ----------------
            emit_dma_in(0)
            for s in range(n_slabs):
                if s + 1 < n_slabs:
                    emit_dma_in(s + 1)
                emit_transp(s, [0, 1, 2, 3, 4])
                if s > 0:
                    emit_pat(s - 1)
                emit_transp(s, [5, 6])
                if s > 0:
                    emit_dec(s - 1)
                emit_p1(s, 0)
                emit_p1(s, 1)
                if s > 0:
                    emit_final(s - 1, [0, 1])
                emit_p2(s)
                if s > 0:
                    emit_final(s - 1, [2, 3])
                    emit_out(s - 1)
                emit_norm(s)
                emit_sims(s)
                emit_topk(s)
            s = n_slabs - 1
            emit_pat(s)
            emit_dec(s)
            emit_final(s, [0, 1, 2, 3])
            emit_out(s)

    nc.finalize()
    return nc


_cache: dict = {}


def _get_nc(rows: int, n_cores: int, bias_mm: bool):
    key = (rows, n_cores, bias_mm)
    if key not in _cache:
        _cache[key] = _build(rows, n_cores, bias_mm)
    return _cache[key]


def kernel(**inputs):
    x = np.ascontiguousarray(np.asarray(inputs["x"], dtype=np.float32))
    rows = x.shape[0]
    n_cores = N_CORES
    rows_pc = rows // n_cores
    bias_mm = not np.allclose(np.asarray(inputs["b4"]), 0.0)
    nc = _get_nc(rows_pc, n_cores, bias_mm)

    w_keys = ["W1", "b1", "W2", "b2", "memory", "W3", "b3", "W4", "b4"]
    weights = {
        k: np.ascontiguousarray(np.asarray(inputs[k], dtype=np.float32))
        for k in w_keys
    }
    in_maps = [
        {"x": x[c * rows_pc:(c + 1) * rows_pc], **weights}
        for c in range(n_cores)
    ]
    res = run_bass_kernel_spmd(
        nc, in_maps, list(range(n_cores)), trace=TRACE
    )
    kernel.last_result = res
    y = np.concatenate([res.results[c]["y"] for c in range(n_cores)], axis=0)
    return y.astype(np.float32)


# revision 20
# speedup vs baseline: 1.6536x; 1.0827x over previous
"""HardAttentionMemoryAE Trainium2 kernel (v2: software-pipelined).

Data-parallel over 8 NeuronCores: x sharded along batch, weights + 50x128
memory bank replicated. Per core the pipeline runs in "transposed
activation" layout (features on partitions, rows on the free dim) so every
matmul contracts along partitions, with a row-major detour for the top-k
masking (per-row ops need rows on partitions).

v2 changes vs v1:
- Emission order software-pipelines slab s's encoder against slab s-1's
  attention/decoder tail so the in-order PE queue never idles (keeps the
  tensor engine p-state at max).
- Top-k thresholding runs on RAW sims (scale-invariant); 1/||z|| is folded
  into the Exp activation's per-partition scale operand.
- Row norms: z row-tiles are transposed on the PE, squared+row-reduced on
  DVE, and 1/sqrt computed with the int-bit-trick + 2 Newton steps on DVE
  (all [128,4] column-layout ops; no serial [1,512] work, no Sqrt table).
- Sigmoid replaced by 0.5*tanh(0.5x)+0.5: tanh/exp/relu/identity/copy all
  live in one activation table set -> zero steady-state ACT_TABLE_LOADs.
  The affine runs on the idle GpSimd(Pool) engine.
- PSUM: 5 rings x {2,2,2,1,1} banks so encoder/decoder/topk phases don't
  serialize on one bank.
"""
import numpy as np
import concourse.bass as bass
import concourse.mybir as mybir
from concourse import bacc
from concourse.tile import TileContext
from concourse.masks import make_identity
from concourse.bass_utils import run_bass_kernel_spmd

F32 = mybir.dt.float32
F32R = mybir.dt.float32r
I32 = mybir.dt.int32
AF = mybir.ActivationFunctionType
ALU = mybir.AluOpType

B_FULL = 65536
D = 784          # input dim
E = 128          # embed dim
M = 50           # memory slots
H = 256          # hidden
N_CORES = 8
SLAB = 512       # rows per slab (4 row-tiles of 128)
NHALF = 392      # final matmul N split (per PSUM bank, >=256 keeps f32r rate)

TRACE = False    # set by test harness for profiling runs

# engine assignment for the 7 per-slab xT PSUM->SBUF copies
XT_COPY_ENG = ["v", "s", "v", "v", "s", "v", "v"]
NEWTON_STEPS = 1


def _build(rows: int, n_cores: int, bias_mm: bool):
    nc = bacc.Bacc(
        "TRN2", target_bir_lowering=False, debug=False,
        enable_asserts=True, num_devices=n_cores
    )
    x = nc.dram_tensor("x", [rows, D], F32, kind="ExternalInput")
    W1 = nc.dram_tensor("W1", [D, H], F32, kind="ExternalInput")
    b1 = nc.dram_tensor("b1", [H], F32, kind="ExternalInput")
    W2 = nc.dram_tensor("W2", [H, E], F32, kind="ExternalInput")
    b2 = nc.dram_tensor("b2", [E], F32, kind="ExternalInput")
    mem = nc.dram_tensor("memory", [M, E], F32, kind="ExternalInput")
    W3 = nc.dram_tensor("W3", [E, H], F32, kind="ExternalInput")
    b3 = nc.dram_tensor("b3", [H], F32, kind="ExternalInput")
    W4 = nc.dram_tensor("W4", [H, D], F32, kind="ExternalInput")
    b4 = nc.dram_tensor("b4", [D], F32, kind="ExternalInput")
    y = nc.dram_tensor("y", [rows, D], F32, kind="ExternalOutput")

    n_slabs = rows // SLAB
    # x col chunks for the transpose: 6 aligned chunks + one overlapping
    # final chunk [656, 784) whose first 112 weight rows are zeroed.
    COFF = [0, 128, 256, 384, 512, 640, D - 128]

    x_r = x[:].rearrange("(s t p) c -> s p t c", p=128, t=4)
    y_r = y[:].rearrange("(s t p) c -> s p t c", p=128, t=4)

    with TileContext(nc) as tc:
        with (
            tc.tile_pool(name="const", bufs=1) as cpool,
            tc.tile_pool(name="xr", bufs=2) as xr_pool,
            tc.tile_pool(name="xT", bufs=2) as xT_pool,
            tc.tile_pool(name="hT", bufs=2) as hT_pool,
            tc.tile_pool(name="zT", bufs=2) as zT_pool,
            tc.tile_pool(name="small", bufs=2) as sm_pool,
            tc.tile_pool(name="xout", bufs=2) as xo_pool,
            tc.tile_pool(name="pbig", bufs=2, space="PSUM") as pbig,
            tc.tile_pool(name="pmid", bufs=2, space="PSUM") as pmid,
            tc.tile_pool(name="pxp", bufs=1, space="PSUM") as pxp,
            tc.tile_pool(name="penc", bufs=1, space="PSUM") as penc,
            tc.tile_pool(name="pmisc", bufs=1, space="PSUM") as pmisc,
        ):
            # ---------------- one-time setup ----------------
            W1sb = cpool.tile([128, 7 * H], F32R)
            zpad = cpool.tile([128, H], F32)
            nc.vector.memset(zpad[:], 0.0)
            nc.scalar.copy(W1sb[:, 6 * H:], zpad[:])
            for c in range(6):
                nc.gpsimd.dma_start(
                    W1sb[:, c * H:(c + 1) * H], W1[COFF[c]:COFF[c] + 128, :]
                )
            nc.gpsimd.dma_start(W1sb[112:128, 6 * H:7 * H], W1[768:D, :])
            W2sb = cpool.tile([128, 2 * E], F32R)
            for m in range(2):
                nc.gpsimd.dma_start(
                    W2sb[:, m * E:(m + 1) * E], W2[m * 128:(m + 1) * 128, :]
                )
            W3sb = cpool.tile([128, H], F32R)
            nc.gpsimd.dma_start(W3sb[:], W3[:])
            W4sb = cpool.tile([128, 2 * D], F32R)
            for k in range(2):
                nc.gpsimd.dma_start(
                    W4sb[:, k * D:(k + 1) * D], W4[k * 128:(k + 1) * 128, :]
                )
            b1sb = cpool.tile([128, 2], F32)
            nc.sync.dma_start(b1sb[:], b1[:].rearrange("(m p) -> p m", p=128))
            b2sb = cpool.tile([128, 1], F32)
            nc.sync.dma_start(b2sb[:], b2[:].rearrange("(p o) -> p o", o=1))
            b3sb = cpool.tile([128, 2], F32)
            nc.sync.dma_start(b3sb[:], b3[:].rearrange("(m p) -> p m", p=128))
            if bias_mm:
                b4row = cpool.tile([1, D], F32R)
                nc.gpsimd.dma_start(b4row[:], b4[:].rearrange("(o c) -> o c", o=1))
                ones_row = cpool.tile([1, 128], F32R)
                onesr_f = cpool.tile([1, 128], F32)
                nc.vector.memset(onesr_f[:], 1.0)
                nc.scalar.copy(ones_row[:], onesr_f[:])

            ident_f = cpool.tile([128, 128], F32)
            make_identity(nc, ident_f[:])
            ident = cpool.tile([128, 128], F32R)
            nc.scalar.copy(ident[:], ident_f[:])

            # normalized memory, transposed: mem_normT [E, M]
            memf = cpool.tile([M, E], F32)
            nc.sync.dma_start(memf[:], mem[:])
            msq = cpool.tile([M, E], F32)
            nc.scalar.square(msq[:], memf[:])
            mss = cpool.tile([M, 1], F32)
            nc.vector.tensor_reduce(mss[:], msq[:], mybir.AxisListType.X, ALU.add)
            nc.scalar.sqrt(mss[:], mss[:])
            nc.vector.tensor_scalar_max(mss[:], mss[:], 1e-12)
            minv = cpool.tile([M, 1], F32)
            nc.vector.reciprocal(minv[:], mss[:])
            mnorm = cpool.tile([M, E], F32R)
            nc.vector.tensor_scalar_mul(mnorm[:], memf[:], minv[:, 0:1])
            p_mn = pmisc.tile([128, 512], F32, tag="misc", name="p_mn")
            nc.tensor.transpose(p_mn[:E, :M].bitcast(F32R), mnorm[:], ident[:M, :M])
            mnT = cpool.tile([E, M], F32R)
            nc.scalar.copy(mnT[:], p_mn[:E, :M])

            # W3m = memory @ W3  [M, H]; decoder contracts attnT against it
            # directly (z_mem never materializes: attn@(mem@W3) == (attn@mem)@W3)
            p_mt = pmisc.tile([128, 512], F32, tag="misc", name="p_mt")
            memr = cpool.tile([M, E], F32R)
            nc.scalar.copy(memr[:], memf[:])
            nc.tensor.transpose(p_mt[:E, :M].bitcast(F32R), memr[:], ident[:M, :M])
            memT = cpool.tile([E, M], F32R)
            nc.scalar.copy(memT[:], p_mt[:E, :M])
            p_w3m = pmisc.tile([128, 512], F32, tag="misc", name="p_w3m")
            nc.tensor.matmul(p_w3m[:M, :H], memT[:], W3sb[:],
                             start=True, stop=True)
            W3msb = cpool.tile([M, H], F32R)
            nc.scalar.copy(W3msb[:], p_w3m[:M, :H])

            # ---------------- per-slab stage emitters ----------------
            st = {}   # slab index -> dict of live tiles

            def copy_eng(which):
                return {"v": nc.vector, "s": nc.scalar, "p": nc.gpsimd}[which]

            def emit_dma_in(s):
                d = st.setdefault(s, {})
                d["xr"] = xr_pool.tile([128, 4, D], F32R, tag="xr",
                                       name=f"xr_{s}")
                nc.gpsimd.dma_start(d["xr"][:], x_r[s])

            def emit_transp(s, chunks):
                d = st[s]
                xT = d.setdefault("xT", {})
                for c in chunks:
                    xT[c] = xT_pool.tile([128, SLAB], F32R, tag=f"xt{c}",
                                         name=f"xt{c}_{s}")
                    ptr = pmid.tile([128, 512], F32, tag="mid",
                                    name=f"ptr{c}_{s}")
                    for t in range(4):
                        nc.tensor.transpose(
                            ptr[:, t * 128:(t + 1) * 128].bitcast(F32R),
                            d["xr"][:, t, COFF[c]:COFF[c] + 128],
                            ident[:],
                        )
                    eng = copy_eng(XT_COPY_ENG[c])
                    if XT_COPY_ENG[c] == "s":
                        nc.scalar.copy(xT[c][:], ptr[:])
                    else:
                        eng.tensor_copy(xT[c][:], ptr[:])

            def emit_p1(s, m):
                d = st[s]
                if "hT" not in d:
                    d["hT"] = hT_pool.tile([128, 1024], F32R, tag="hT",
                                           name=f"hT_{s}")
                ph = pbig.tile([128, 512], F32, tag="big", name=f"ph{m}_{s}")
                for c in range(7):
                    nc.tensor.matmul(
                        ph[:],
                        W1sb[:, c * H + m * 128: c * H + m * 128 + 128],
                        d["xT"][c][:],
                        start=(c == 0), stop=(c == 6),
                    )
                nc.scalar.activation(
                    d["hT"][:, m * 512:(m + 1) * 512], ph[:],
                    AF.Relu, bias=b1sb[:, m:m + 1],
                )

            def emit_p2(s):
                d = st[s]
                pz = penc.tile([128, 512], F32, tag="enc", name=f"pz_{s}")
                for m in range(2):
                    nc.tensor.matmul(
                        pz[:], W2sb[:, m * E:(m + 1) * E],
                        d["hT"][:, m * 512:(m + 1) * 512],
                        start=(m == 0), stop=(m == 1),
                    )
                d["zT"] = zT_pool.tile([128, SLAB], F32R, tag="zT",
                                       name=f"zT_{s}")
                nc.scalar.activation(d["zT"][:], pz[:], AF.Identity,
                                     bias=b2sb[:, 0:1])

            def emit_norm(s):
                # row norms in column layout: transpose z row-tiles on PE,
                # square+reduce on DVE, rsqrt via bit trick + 2 Newton steps.
                d = st[s]
                zrm = pmisc.tile([128, 512], F32, tag="misc", name=f"zrm_{s}")
                for t in range(4):
                    nc.tensor.transpose(
                        zrm[:, t * 128:(t + 1) * 128].bitcast(F32R),
                        d["zT"][:, t * 128:(t + 1) * 128],
                        ident[:],
                    )
                zsqc = sm_pool.tile([128, 512], F32, tag="zsqc",
                                    name=f"zsqc_{s}")
                nc.scalar.square(zsqc[:], zrm[:])
                nsq = sm_pool.tile([128, 4], F32, tag="nsq", name=f"nsq_{s}")
                nc.vector.tensor_reduce(
                    nsq[:], zsqc[:].rearrange("p (t c) -> p t c", c=128),
                    mybir.AxisListType.X, ALU.add,
                )
                # inv = 1/sqrt(nsq): magic-constant seed + 2 Newton steps
                seed_i = sm_pool.tile([128, 4], I32, tag="seed_i",
                                      name=f"seed_i_{s}")
                nc.vector.tensor_scalar(
                    out=seed_i[:], in0=nsq[:].bitcast(I32),
                    scalar1=1, scalar2=None, op0=ALU.logical_shift_right,
                )
                y0_i = sm_pool.tile([128, 4], I32, tag="y0_i",
                                    name=f"y0_i_{s}")
                nc.vector.tensor_scalar(
                    out=y0_i[:], in0=seed_i[:],
                    scalar1=-1, scalar2=0x5F3759DF, op0=ALU.mult,
                    op1=ALU.add,
                )
                # y0_i = 0x5f3759df - (bits(nsq) >> 1): rsqrt seed
                h = sm_pool.tile([128, 4], F32, tag="h", name=f"h_{s}")
                nc.vector.tensor_scalar(
                    out=h[:], in0=nsq[:], scalar1=0.5, scalar2=1e-30,
                    op0=ALU.mult, op1=ALU.max,
                )
                ycur = y0_i[:].bitcast(F32)
                for it in range(NEWTON_STEPS):
                    a = sm_pool.tile([128, 4], F32, tag=f"nta{it}",
                                     name=f"nta{it}_{s}")
                    nc.vector.tensor_tensor(a[:], ycur, ycur, ALU.mult)
                    b_ = sm_pool.tile([128, 4], F32, tag=f"ntb{it}",
                                      name=f"ntb{it}_{s}")
                    nc.vector.tensor_tensor(b_[:], a[:], h[:], ALU.mult)
                    c_ = sm_pool.tile([128, 4], F32, tag=f"ntc{it}",
                                      name=f"ntc{it}_{s}")
                    nc.vector.tensor_scalar(
                        out=c_[:], in0=b_[:], scalar1=-1.0, scalar2=1.5,
                        op0=ALU.mult, op1=ALU.add,
                    )
                    ynext = sm_pool.tile([128, 4], F32, tag=f"nty{it}",
                                         name=f"nty{it}_{s}")
                    nc.vector.tensor_tensor(ynext[:], ycur, c_[:], ALU.mult)
                    ycur = ynext[:]
                d["invcol"] = ycur

            def emit_sims(s):
                d = st[s]
                psim = penc.tile([128, 512], F32, tag="enc", name=f"psim_{s}")
                for t in range(4):
                    nc.tensor.matmul(
                        psim[:, t * M:(t + 1) * M],
                        d["zT"][:, t * 128:(t + 1) * 128], mnT[:],
                        start=True, stop=True,
                    )
                d["psim"] = psim

            def emit_topk(s):
                d = st[s]
                simsb = sm_pool.tile([128, 4 * M], F32, tag="simsb",
                                     name=f"simsb_{s}")
                nc.vector.tensor_copy(simsb[:], d["psim"][:, :4 * M])
                m8 = sm_pool.tile([128, 32], F32, tag="m8", name=f"m8_{s}")
                msk = sm_pool.tile([128, 4 * M], F32, tag="msk",
                                   name=f"msk_{s}")
                pexp = sm_pool.tile([128, 4 * M], F32, tag="pexp",
                                    name=f"pexp_{s}")
                den = sm_pool.tile([128, 4], F32, tag="den", name=f"den_{s}")
                for t in range(4):
                    nc.vector.max(m8[:, t * 8:(t + 1) * 8],
                                  simsb[:, t * M:(t + 1) * M])
                    nc.vector.scalar_tensor_tensor(
                        out=msk[:, t * M:(t + 1) * M],
                        in0=simsb[:, t * M:(t + 1) * M],
                        scalar=m8[:, t * 8 + 4:t * 8 + 5],
                        in1=simsb[:, t * M:(t + 1) * M],
                        op0=ALU.is_ge, op1=ALU.mult,
                    )
                    nc.scalar.activation(
                        pexp[:, t * M:(t + 1) * M], msk[:, t * M:(t + 1) * M],
                        AF.Exp, scale=d["invcol"][:, t:t + 1],
                        accum_out=den[:, t:t + 1],
                    )
                rden = sm_pool.tile([128, 4], F32, tag="rden",
                                    name=f"rden_{s}")
                nc.vector.reciprocal(rden[:], den[:])
                attn = sm_pool.tile([128, 4 * M], F32R, tag="attn",
                                    name=f"attn_{s}")
                for t in range(4):
                    nc.vector.tensor_scalar_mul(
                        attn[:, t * M:(t + 1) * M],
                        pexp[:, t * M:(t + 1) * M],
                        rden[:, t:t + 1],
                    )
                d["attn"] = attn

            def emit_pat(s):
                d = st[s]
                pat = pmisc.tile([128, 512], F32, tag="misc", name=f"pat_{s}")
                for t in range(4):
                    nc.tensor.transpose(
                        pat[:M, t * 128:(t + 1) * 128].bitcast(F32R),
                        d["attn"][:, t * M:(t + 1) * M], ident[:],
                    )
                attnT = sm_pool.tile([M, SLAB], F32R, tag="attnT",
                                     name=f"attnT_{s}")
                nc.vector.tensor_copy(attnT[:], pat[:M, :])
                d["attnT"] = attnT

            def emit_dec(s):
                d = st[s]
                d["dT"] = hT_pool.tile([128, 1024], F32R, tag="dT",
                                       name=f"dT_{s}")
                for m in range(2):
                    pd = pbig.tile([128, 512], F32, tag="big",
                                   name=f"pd{m}_{s}")
                    nc.tensor.matmul(
                        pd[:], W3msb[:, m * 128:(m + 1) * 128], d["attnT"][:],
                        start=True, stop=True,
                    )
                    nc.vector.tensor_scalar(
                        out=d["dT"][:, m * 512:(m + 1) * 512], in0=pd[:],
                        scalar1=b3sb[:, m:m + 1], scalar2=0.0,
                        op0=ALU.add, op1=ALU.max,
                    )

            def emit_final(s, tiles):
                d = st[s]
                if "xo" not in d:
                    d["xo"] = xo_pool.tile([128, 4, D], F32, tag="xo",
                                           name=f"xo_{s}")
                for t in tiles:
                    px = pxp.tile([128, 1024], F32, tag="x",
                                  name=f"px{t}_{s}")
                    for nh in range(2):
                        pxh = px[:, nh * 512:nh * 512 + NHALF]
                        if bias_mm:
                            nc.tensor.matmul(
                                pxh, ones_row[:],
                                b4row[:, nh * NHALF:(nh + 1) * NHALF],
                                start=True, stop=False,
                            )
                        for k in range(2):
                            nc.tensor.matmul(
                                pxh,
                                d["dT"][:, k * 512 + t * 128:
                                        k * 512 + t * 128 + 128],
                                W4sb[:, k * D + nh * NHALF:
                                     k * D + (nh + 1) * NHALF],
                                start=(k == 0 and not bias_mm), stop=(k == 1),
                            )
                    # sigmoid(v) = 0.5*tanh(0.5 v)+0.5; one 3D-AP tanh per
                    # row tile, affine fixup on the (otherwise idle) pool
                    pxv = px[:].rearrange("p (n c) -> p n c", n=2)[:, :, :NHALF]
                    xov = d["xo"][:, t, :].rearrange("p (n c) -> p n c", n=2)
                    nc.scalar.activation(xov, pxv, AF.Tanh, scale=0.5)
                    nc.gpsimd.tensor_scalar(
                        out=d["xo"][:, t, :], in0=d["xo"][:, t, :],
                        scalar1=0.5, scalar2=0.5,
                        op0=ALU.mult, op1=ALU.add,
                    )

            def emit_out(s):
                nc.sync.dma_start(y_r[s], st[s]["xo"][:])
                # drop references to this slab's tiles
                del st[s]

            # ---------------- software-pipelined slab loop ----------------
            emit_dma_in(0)
            for s in range(n_slabs):
                if s + 1 < n_slabs:
                    emit_dma_in(s + 1)
                emit_transp(s, [0, 1, 2, 3, 4, 5, 6])
                if s > 0:
                    emit_topk(s - 1)
                emit_p1(s, 0)
                if s > 0:
                    emit_pat(s - 1)
                    emit_dec(s - 1)
                emit_p1(s, 1)
                if s > 0:
                    emit_final(s - 1, [0, 1])
                emit_p2(s)
                if s > 0:
                    emit_final(s - 1, [2, 3])
                    emit_out(s - 1)
                emit_norm(s)
                emit_sims(s)
            s = n_slabs - 1
            emit_topk(s)
            emit_pat(s)
            emit_dec(s)
            emit_final(s, [0, 1, 2, 3])
            emit_out(s)

    nc.finalize()
    return nc


_cache: dict = {}


def _get_nc(rows: int, n_cores: int, bias_mm: bool):
    key = (rows, n_cores, bias_mm)
    if key not in _cache:
        _cache[key] = _build(rows, n_cores, bias_mm)
    return _cache[key]


def kernel(**inputs):
    x = np.ascontiguousarray(np.asarray(inputs["x"], dtype=np.float32))
    rows = x.shape[0]
    n_cores = N_CORES
    rows_pc = rows // n_cores
    bias_mm = not np.allclose(np.asarray(inputs["b4"]), 0.0)
    nc = _get_nc(rows_pc, n_cores, bias_mm)

    w_keys = ["W1", "b1", "W2", "b2", "memory", "W3", "b3", "W4", "b4"]
    weights = {
        k: np.ascontiguousarray(np.asarray(inputs[k], dtype=np.float32))
        for k in w_keys
    }
    in_maps = [
        {"x": x[c * rows_pc:(c + 1) * rows_pc], **weights}
        for c in range(n_cores)
    ]
    res = run_bass_kernel_spmd(
        nc, in_maps, list(range(n_cores)), trace=TRACE
    )
    kernel.last_result = res
    y = np.concatenate([res.results[c]["y"] for c in range(n_cores)], axis=0)
    return y.astype(np.float32)


# revision 22
# speedup vs baseline: 1.6550x; 1.0009x over previous
"""HardAttentionMemoryAE Trainium2 kernel (v2: software-pipelined).

Data-parallel over 8 NeuronCores: x sharded along batch, weights + 50x128
memory bank replicated. Per core the pipeline runs in "transposed
activation" layout (features on partitions, rows on the free dim) so every
matmul contracts along partitions, with a row-major detour for the top-k
masking (per-row ops need rows on partitions).

v2 changes vs v1:
- Emission order software-pipelines slab s's encoder against slab s-1's
  attention/decoder tail so the in-order PE queue never idles (keeps the
  tensor engine p-state at max).
- Top-k thresholding runs on RAW sims (scale-invariant); 1/||z|| is folded
  into the Exp activation's per-partition scale operand.
- Row norms: z row-tiles are transposed on the PE, squared+row-reduced on
  DVE, and 1/sqrt computed with the int-bit-trick + 2 Newton steps on DVE
  (all [128,4] column-layout ops; no serial [1,512] work, no Sqrt table).
- Sigmoid replaced by 0.5*tanh(0.5x)+0.5: tanh/exp/relu/identity/copy all
  live in one activation table set -> zero steady-state ACT_TABLE_LOADs.
  The affine runs on the idle GpSimd(Pool) engine.
- PSUM: 5 rings x {2,2,2,1,1} banks so encoder/decoder/topk phases don't
  serialize on one bank.
"""
import numpy as np
import concourse.bass as bass
import concourse.mybir as mybir
from concourse import bacc
from concourse.tile import TileContext
from concourse.masks import make_identity
from concourse.bass_utils import run_bass_kernel_spmd

F32 = mybir.dt.float32
F32R = mybir.dt.float32r
I32 = mybir.dt.int32
AF = mybir.ActivationFunctionType
ALU = mybir.AluOpType

B_FULL = 65536
D = 784          # input dim
E = 128          # embed dim
M = 50           # memory slots
H = 256          # hidden
N_CORES = 8
SLAB = 512       # rows per slab (4 row-tiles of 128)
NHALF = 392      # final matmul N split (per PSUM bank, >=256 keeps f32r rate)

TRACE = False    # set by test harness for profiling runs

# engine assignment for the 7 per-slab xT PSUM->SBUF copies
XT_COPY_ENG = ["v", "s", "v", "v", "s", "v", "v"]
NEWTON_STEPS = 1


def _build(rows: int, n_cores: int, bias_mm: bool):
    nc = bacc.Bacc(
        "TRN2", target_bir_lowering=False, debug=False,
        enable_asserts=True, num_devices=n_cores
    )
    x = nc.dram_tensor("x", [rows, D], F32, kind="ExternalInput")
    W1 = nc.dram_tensor("W1", [D, H], F32, kind="ExternalInput")
    b1 = nc.dram_tensor("b1", [H], F32, kind="ExternalInput")
    W2 = nc.dram_tensor("W2", [H, E], F32, kind="ExternalInput")
    b2 = nc.dram_tensor("b2", [E], F32, kind="ExternalInput")
    mem = nc.dram_tensor("memory", [M, E], F32, kind="ExternalInput")
    W3 = nc.dram_tensor("W3", [E, H], F32, kind="ExternalInput")
    b3 = nc.dram_tensor("b3", [H], F32, kind="ExternalInput")
    W4 = nc.dram_tensor("W4", [H, D], F32, kind="ExternalInput")
    b4 = nc.dram_tensor("b4", [D], F32, kind="ExternalInput")
    y = nc.dram_tensor("y", [rows, D], F32, kind="ExternalOutput")

    n_slabs = rows // SLAB
    # x col chunks for the transpose: 6 aligned chunks + one overlapping
    # final chunk [656, 784) whose first 112 weight rows are zeroed.
    COFF = [0, 128, 256, 384, 512, 640, D - 128]

    x_r = x[:].rearrange("(s t p) c -> s p t c", p=128, t=4)
    y_r = y[:].rearrange("(s t p) c -> s p t c", p=128, t=4)

    with TileContext(nc) as tc:
        with (
            tc.tile_pool(name="const", bufs=1) as cpool,
            tc.tile_pool(name="xr", bufs=2) as xr_pool,
            tc.tile_pool(name="xT", bufs=2) as xT_pool,
            tc.tile_pool(name="hT", bufs=2) as hT_pool,
            tc.tile_pool(name="zT", bufs=2) as zT_pool,
            tc.tile_pool(name="small", bufs=2) as sm_pool,
            tc.tile_pool(name="xout", bufs=2) as xo_pool,
            tc.tile_pool(name="pbig", bufs=2, space="PSUM") as pbig,
            tc.tile_pool(name="pmid", bufs=2, space="PSUM") as pmid,
            tc.tile_pool(name="pxp", bufs=1, space="PSUM") as pxp,
            tc.tile_pool(name="penc", bufs=1, space="PSUM") as penc,
            tc.tile_pool(name="pmisc", bufs=1, space="PSUM") as pmisc,
        ):
            # ---------------- one-time setup ----------------
            W1sb = cpool.tile([128, 7 * H], F32R)
            zpad = cpool.tile([128, H], F32)
            nc.vector.memset(zpad[:], 0.0)
            nc.scalar.copy(W1sb[:, 6 * H:], zpad[:])
            for c in range(6):
                nc.gpsimd.dma_start(
                    W1sb[:, c * H:(c + 1) * H], W1[COFF[c]:COFF[c] + 128, :]
                )
            nc.gpsimd.dma_start(W1sb[112:128, 6 * H:7 * H], W1[768:D, :])
            W2sb = cpool.tile([128, 2 * E], F32R)
            for m in range(2):
                nc.gpsimd.dma_start(
                    W2sb[:, m * E:(m + 1) * E], W2[m * 128:(m + 1) * 128, :]
                )
            W3sb = cpool.tile([128, H], F32R)
            nc.gpsimd.dma_start(W3sb[:], W3[:])
            W4sb = cpool.tile([128, 2 * D], F32R)
            for k in range(2):
                nc.gpsimd.dma_start(
                    W4sb[:, k * D:(k + 1) * D], W4[k * 128:(k + 1) * 128, :]
                )
            b1sb = cpool.tile([128, 2], F32)
            nc.sync.dma_start(b1sb[:], b1[:].rearrange("(m p) -> p m", p=128))
            b2sb = cpool.tile([128, 1], F32)
            nc.sync.dma_start(b2sb[:], b2[:].rearrange("(p o) -> p o", o=1))
            b3sb = cpool.tile([128, 2], F32)
            nc.sync.dma_start(b3sb[:], b3[:].rearrange("(m p) -> p m", p=128))
            if bias_mm:
                b4row = cpool.tile([1, D], F32R)
                nc.gpsimd.dma_start(b4row[:], b4[:].rearrange("(o c) -> o c", o=1))
                ones_row = cpool.tile([1, 128], F32R)
                onesr_f = cpool.tile([1, 128], F32)
                nc.vector.memset(onesr_f[:], 1.0)
                nc.scalar.copy(ones_row[:], onesr_f[:])

            ident_f = cpool.tile([128, 128], F32)
            make_identity(nc, ident_f[:])
            ident = cpool.tile([128, 128], F32R)
            nc.scalar.copy(ident[:], ident_f[:])

            # normalized memory, transposed: mem_normT [E, M]
            memf = cpool.tile([M, E], F32)
            nc.sync.dma_start(memf[:], mem[:])
            msq = cpool.tile([M, E], F32)
            nc.scalar.square(msq[:], memf[:])
            mss = cpool.tile([M, 1], F32)
            nc.vector.tensor_reduce(mss[:], msq[:], mybir.AxisListType.X, ALU.add)
            nc.scalar.sqrt(mss[:], mss[:])
            nc.vector.tensor_scalar_max(mss[:], mss[:], 1e-12)
            minv = cpool.tile([M, 1], F32)
            nc.vector.reciprocal(minv[:], mss[:])
            mnorm = cpool.tile([M, E], F32R)
            nc.vector.tensor_scalar_mul(mnorm[:], memf[:], minv[:, 0:1])
            p_mn = pmisc.tile([128, 512], F32, tag="misc", name="p_mn")
            nc.tensor.transpose(p_mn[:E, :M].bitcast(F32R), mnorm[:], ident[:M, :M])
            mnT = cpool.tile([E, M], F32R)
            nc.scalar.copy(mnT[:], p_mn[:E, :M])

            # W3m = memory @ W3  [M, H]; decoder contracts attnT against it
            # directly (z_mem never materializes: attn@(mem@W3) == (attn@mem)@W3)
            p_mt = pmisc.tile([128, 512], F32, tag="misc", name="p_mt")
            memr = cpool.tile([M, E], F32R)
            nc.scalar.copy(memr[:], memf[:])
            nc.tensor.transpose(p_mt[:E, :M].bitcast(F32R), memr[:], ident[:M, :M])
            memT = cpool.tile([E, M], F32R)
            nc.scalar.copy(memT[:], p_mt[:E, :M])
            p_w3m = pmisc.tile([128, 512], F32, tag="misc", name="p_w3m")
            nc.tensor.matmul(p_w3m[:M, :H], memT[:], W3sb[:],
                             start=True, stop=True)
            W3msb = cpool.tile([M, H], F32R)
            nc.scalar.copy(W3msb[:], p_w3m[:M, :H])

            # ---------------- per-slab stage emitters ----------------
            st = {}   # slab index -> dict of live tiles

            def copy_eng(which):
                return {"v": nc.vector, "s": nc.scalar, "p": nc.gpsimd}[which]

            def emit_dma_in(s):
                d = st.setdefault(s, {})
                d["xr"] = xr_pool.tile([128, 4, D], F32R, tag="xr",
                                       name=f"xr_{s}")
                nc.gpsimd.dma_start(d["xr"][:], x_r[s])

            def emit_transp(s, chunks):
                d = st[s]
                xT = d.setdefault("xT", {})
                for c in chunks:
                    xT[c] = xT_pool.tile([128, SLAB], F32R, tag=f"xt{c}",
                                         name=f"xt{c}_{s}")
                    ptr = pmid.tile([128, 512], F32, tag="mid",
                                    name=f"ptr{c}_{s}")
                    for t in range(4):
                        nc.tensor.transpose(
                            ptr[:, t * 128:(t + 1) * 128].bitcast(F32R),
                            d["xr"][:, t, COFF[c]:COFF[c] + 128],
                            ident[:],
                        )
                    eng = copy_eng(XT_COPY_ENG[c])
                    if XT_COPY_ENG[c] == "s":
                        nc.scalar.copy(xT[c][:], ptr[:])
                    else:
                        eng.tensor_copy(xT[c][:], ptr[:])

            def emit_p1(s, m):
                d = st[s]
                if "hT" not in d:
                    d["hT"] = hT_pool.tile([128, 1024], F32R, tag="hT",
                                           name=f"hT_{s}")
                ph = pbig.tile([128, 512], F32, tag="big", name=f"ph{m}_{s}")
                for c in range(7):
                    nc.tensor.matmul(
                        ph[:],
                        W1sb[:, c * H + m * 128: c * H + m * 128 + 128],
                        d["xT"][c][:],
                        start=(c == 0), stop=(c == 6),
                    )
                nc.scalar.activation(
                    d["hT"][:, m * 512:(m + 1) * 512], ph[:],
                    AF.Relu, bias=b1sb[:, m:m + 1],
                )

            def emit_p2(s):
                d = st[s]
                pz = penc.tile([128, 512], F32, tag="enc", name=f"pz_{s}")
                for m in range(2):
                    nc.tensor.matmul(
                        pz[:], W2sb[:, m * E:(m + 1) * E],
                        d["hT"][:, m * 512:(m + 1) * 512],
                        start=(m == 0), stop=(m == 1),
                    )
                d["zT"] = zT_pool.tile([128, SLAB], F32R, tag="zT",
                                       name=f"zT_{s}")
                nc.scalar.activation(d["zT"][:], pz[:], AF.Identity,
                                     bias=b2sb[:, 0:1])

            def emit_norm(s):
                # row norms in column layout: transpose z row-tiles on PE,
                # square+reduce on DVE, rsqrt via bit trick + 2 Newton steps.
                d = st[s]
                zrm = pmisc.tile([128, 512], F32, tag="misc", name=f"zrm_{s}")
                for t in range(4):
                    nc.tensor.transpose(
                        zrm[:, t * 128:(t + 1) * 128].bitcast(F32R),
                        d["zT"][:, t * 128:(t + 1) * 128],
                        ident[:],
                    )
                zsqc = sm_pool.tile([128, 512], F32, tag="zsqc",
                                    name=f"zsqc_{s}")
                nc.scalar.square(zsqc[:], zrm[:])
                nsq = sm_pool.tile([128, 4], F32, tag="nsq", name=f"nsq_{s}")
                nc.vector.tensor_reduce(
                    nsq[:], zsqc[:].rearrange("p (t c) -> p t c", c=128),
                    mybir.AxisListType.X, ALU.add,
                )
                # inv = 1/sqrt(nsq): magic-constant seed + 2 Newton steps
                seed_i = sm_pool.tile([128, 4], I32, tag="seed_i",
                                      name=f"seed_i_{s}")
                nc.vector.tensor_scalar(
                    out=seed_i[:], in0=nsq[:].bitcast(I32),
                    scalar1=1, scalar2=None, op0=ALU.logical_shift_right,
                )
                y0_i = sm_pool.tile([128, 4], I32, tag="y0_i",
                                    name=f"y0_i_{s}")
                nc.vector.tensor_scalar(
                    out=y0_i[:], in0=seed_i[:],
                    scalar1=-1, scalar2=0x5F3759DF, op0=ALU.mult,
                    op1=ALU.add,
                )
                # y0_i = 0x5f3759df - (bits(nsq) >> 1): rsqrt seed
                h = sm_pool.tile([128, 4], F32, tag="h", name=f"h_{s}")
                nc.gpsimd.tensor_scalar(
                    out=h[:], in0=nsq[:], scalar1=0.5, scalar2=1e-30,
                    op0=ALU.mult, op1=ALU.max,
                )
                ycur = y0_i[:].bitcast(F32)
                for it in range(NEWTON_STEPS):
                    a = sm_pool.tile([128, 4], F32, tag=f"nta{it}",
                                     name=f"nta{it}_{s}")
                    nc.gpsimd.tensor_tensor(a[:], ycur, ycur, ALU.mult)
                    b_ = sm_pool.tile([128, 4], F32, tag=f"ntb{it}",
                                      name=f"ntb{it}_{s}")
                    nc.gpsimd.tensor_tensor(b_[:], a[:], h[:], ALU.mult)
                    c_ = sm_pool.tile([128, 4], F32, tag=f"ntc{it}",
                                      name=f"ntc{it}_{s}")
                    nc.gpsimd.tensor_scalar(
                        out=c_[:], in0=b_[:], scalar1=-1.0, scalar2=1.5,
                        op0=ALU.mult, op1=ALU.add,
                    )
                    ynext = sm_pool.tile([128, 4], F32, tag=f"nty{it}",
                                         name=f"nty{it}_{s}")
                    nc.gpsimd.tensor_tensor(ynext[:], ycur, c_[:], ALU.mult)
                    ycur = ynext[:]
                d["invcol"] = ycur

            def emit_sims(s):
                d = st[s]
                psim = penc.tile([128, 512], F32, tag="enc", name=f"psim_{s}")
                for t in range(4):
                    nc.tensor.matmul(
                        psim[:, t * M:(t + 1) * M],
                        d["zT"][:, t * 128:(t + 1) * 128], mnT[:],
                        start=True, stop=True,
                    )
                d["psim"] = psim

            def emit_topk(s):
                d = st[s]
                simsb = sm_pool.tile([128, 4 * M], F32, tag="simsb",
                                     name=f"simsb_{s}")
                nc.vector.tensor_copy(simsb[:], d["psim"][:, :4 * M])
                m8 = sm_pool.tile([128, 32], F32, tag="m8", name=f"m8_{s}")
                msk = sm_pool.tile([128, 4 * M], F32, tag="msk",
                                   name=f"msk_{s}")
                pexp = sm_pool.tile([128, 4 * M], F32, tag="pexp",
                                    name=f"pexp_{s}")
                den = sm_pool.tile([128, 4], F32, tag="den", name=f"den_{s}")
                for t in range(4):
                    nc.vector.max(m8[:, t * 8:(t + 1) * 8],
                                  simsb[:, t * M:(t + 1) * M])
                    nc.vector.scalar_tensor_tensor(
                        out=msk[:, t * M:(t + 1) * M],
                        in0=simsb[:, t * M:(t + 1) * M],
                        scalar=m8[:, t * 8 + 4:t * 8 + 5],
                        in1=simsb[:, t * M:(t + 1) * M],
                        op0=ALU.is_ge, op1=ALU.mult,
                    )
                    nc.scalar.activation(
                        pexp[:, t * M:(t + 1) * M], msk[:, t * M:(t + 1) * M],
                        AF.Exp, scale=d["invcol"][:, t:t + 1],
                        accum_out=den[:, t:t + 1],
                    )
                rden = sm_pool.tile([128, 4], F32, tag="rden",
                                    name=f"rden_{s}")
                nc.vector.reciprocal(rden[:], den[:])
                attn = sm_pool.tile([128, 4 * M], F32R, tag="attn",
                                    name=f"attn_{s}")
                for t in range(4):
                    nc.vector.tensor_scalar_mul(
                        attn[:, t * M:(t + 1) * M],
                        pexp[:, t * M:(t + 1) * M],
                        rden[:, t:t + 1],
                    )
                d["attn"] = attn

            def emit_pat(s):
                d = st[s]
                pat = pmisc.tile([128, 512], F32, tag="misc", name=f"pat_{s}")
                for t in range(4):
                    nc.tensor.transpose(
                        pat[:M, t * 128:(t + 1) * 128].bitcast(F32R),
                        d["attn"][:, t * M:(t + 1) * M], ident[:],
                    )
                attnT = sm_pool.tile([M, SLAB], F32R, tag="attnT",
                                     name=f"attnT_{s}")
                nc.vector.tensor_copy(attnT[:], pat[:M, :])
                d["attnT"] = attnT

            def emit_dec(s):
                d = st[s]
                d["dT"] = hT_pool.tile([128, 1024], F32R, tag="dT",
                                       name=f"dT_{s}")
                for m in range(2):
                    pd = pbig.tile([128, 512], F32, tag="big",
                                   name=f"pd{m}_{s}")
                    nc.tensor.matmul(
                        pd[:], W3msb[:, m * 128:(m + 1) * 128], d["attnT"][:],
                        start=True, stop=True,
                    )
                    nc.vector.tensor_scalar(
                        out=d["dT"][:, m * 512:(m + 1) * 512], in0=pd[:],
                        scalar1=b3sb[:, m:m + 1], scalar2=0.0,
                        op0=ALU.add, op1=ALU.max,
                    )

            def emit_final(s, tiles):
                d = st[s]
                if "xo" not in d:
                    d["xo"] = xo_pool.tile([128, 4, D], F32, tag="xo",
                                           name=f"xo_{s}")
                for t in tiles:
                    px = pxp.tile([128, 1024], F32, tag="x",
                                  name=f"px{t}_{s}")
                    for nh in range(2):
                        pxh = px[:, nh * 512:nh * 512 + NHALF]
                        if bias_mm:
                            nc.tensor.matmul(
                                pxh, ones_row[:],
                                b4row[:, nh * NHALF:(nh + 1) * NHALF],
                                start=True, stop=False,
                            )
                        for k in range(2):
                            nc.tensor.matmul(
                                pxh,
                                d["dT"][:, k * 512 + t * 128:
                                        k * 512 + t * 128 + 128],
                                W4sb[:, k * D + nh * NHALF:
                                     k * D + (nh + 1) * NHALF],
                                start=(k == 0 and not bias_mm), stop=(k == 1),
                            )
                    # sigmoid(v) = 0.5*tanh(0.5 v)+0.5; one 3D-AP tanh per
                    # row tile, affine fixup on the (otherwise idle) pool
                    pxv = px[:].rearrange("p (n c) -> p n c", n=2)[:, :, :NHALF]
                    xov = d["xo"][:, t, :].rearrange("p (n c) -> p n c", n=2)
                    nc.scalar.activation(xov, pxv, AF.Tanh, scale=0.5)
                    nc.gpsimd.tensor_scalar(
                        out=d["xo"][:, t, :], in0=d["xo"][:, t, :],
                        scalar1=0.5, scalar2=0.5,
                        op0=ALU.mult, op1=ALU.add,
                    )

            def emit_out(s):
                nc.sync.dma_start(y_r[s], st[s]["xo"][:])
                # drop references to this slab's tiles
                del st[s]

            # ---------------- software-pipelined slab loop ----------------
            emit_dma_in(0)
            for s in range(n_slabs):
                if s + 1 < n_slabs:
                    emit_dma_in(s + 1)
                emit_transp(s, [0, 1, 2, 3, 4])
                if s > 0:
                    emit_topk(s - 1)
                emit_transp(s, [5, 6])
                emit_p1(s, 0)
                if s > 0:
                    emit_pat(s - 1)
                    emit_dec(s - 1)
                emit_p1(s, 1)
                if s > 0:
                    emit_final(s - 1, [0, 1])
                emit_p2(s)
                emit_norm(s)
                emit_sims(s)
                if s > 0:
                    emit_final(s - 1, [2, 3])
                    emit_out(s - 1)
            s = n_slabs - 1
            emit_topk(s)
            emit_pat(s)
            emit_dec(s)
            emit_final(s, [0, 1, 2, 3])
            emit_out(s)

    nc.finalize()
    return nc


_cache: dict = {}


def _get_nc(rows: int, n_cores: int, bias_mm: bool):
    key = (rows, n_cores, bias_mm)
    if key not in _cache:
        _cache[key] = _build(rows, n_cores, bias_mm)
    return _cache[key]


def kernel(**inputs):
    x = np.ascontiguousarray(np.asarray(inputs["x"], dtype=np.float32))
    rows = x.shape[0]
    n_cores = N_CORES
    rows_pc = rows // n_cores
    bias_mm = not np.allclose(np.asarray(inputs["b4"]), 0.0)
    nc = _get_nc(rows_pc, n_cores, bias_mm)

    w_keys = ["W1", "b1", "W2", "b2", "memory", "W3", "b3", "W4", "b4"]
    weights = {
        k: np.ascontiguousarray(np.asarray(inputs[k], dtype=np.float32))
        for k in w_keys
    }
    in_maps = [
        {"x": x[c * rows_pc:(c + 1) * rows_pc], **weights}
        for c in range(n_cores)
    ]
    res = run_bass_kernel_spmd(
        nc, in_maps, list(range(n_cores)), trace=TRACE
    )
    kernel.last_result = res
    y = np.concatenate([res.results[c]["y"] for c in range(n_cores)], axis=0)
    return y.astype(np.float32)
